# revision 1
# baseline (speedup 1.0000x reference)
"""MoE logistic regression kernel for 8 Trainium2 NeuronCores.

Math (after dead-code elimination of the reference's unused router path):
    noise_logits = x @ noise_w.T + noise_b            # [B, E]
    top8 = top_k(noise_logits, 8)
    gates = softmax over the top-8 entries (others 0)
    expert = sigmoid(x @ expert_w.T + expert_b)       # [B, E]
    out[b] = sum_e gates[b,e] * expert[b,e]           # [B, 1]

Sharding: batch split 8 ways (2048 rows/core); weights replicated.

Implementation notes:
- Single-pass fp16 matmul (x, w rounded on host). Logit error ~2.3e-4
  flips the 8th/9th expert on ~24/16384 rows; end-to-end l2 rel err
  ~1.2e-3 vs the 2e-2 gate, at half the DMA and a third of the PE work
  of an fp16 hi/lo split. The x stream is the roofline: ~46.6us of
  gapless DMA at the HBM limit.
- Batch-tile-major stream: each tile's full contraction arrives while
  the previous tile's epilogue runs on ACT/DVE. Tile widths taper
  (512,512,512,256,128,128) and the last tile's DMA groups taper too,
  so the serial tail after the last byte is one short epilogue.
- The epilogue never uses the ACT sigmoid table: sigmoid comes from
  exp(-z) + DVE 1/(1+e), and softmax skips the max-shift (logits are
  bounded ~|4|), so every ACT op stays in the one exp_and_others
  function set -- no mid-stream LoadActFuncSet (1.3us each).
- Top-8 gating via DVE Max8 + fused (e >= e8) mask * e with accumulated
  row sum (scalar_tensor_tensor), all on the SBUF exp(v) copy --
  exp is monotone so selection is identical, and avoiding a second
  PSUM reader dodges cross-engine read serialization.
- Per-tile outputs DMA straight from the [128, j] result (no final
  transpose); early tiles go via the idle gpsimd SWDGE path so they
  never head-of-line-block the x stream on the SP HWDGE queue.
"""

import sys

import numpy as np

if "/opt/trn_rl_repo" not in sys.path:
    sys.path.insert(0, "/opt/trn_rl_repo")

B, D, E, TOPK, NCORES = 16384, 4096, 64, 8, 8
BC = B // NCORES      # batch rows per core
NK = D // 128         # contraction chunks
TILES = [512, 512, 512, 256, 128, 128]          # batch tile widths
OFFS = [sum(TILES[:i]) for i in range(len(TILES))]
assert sum(TILES) == BC
# DMA grouping in k-chunks per tile; the final tile tapers so almost no
# matmul work remains after the last byte lands
GROUPS = [[8, 8, 8, 8]] * 5 + [[16, 8, 4, 2, 2]]

_cached = {}


def _build_program():
    import concourse.bass as bass
    import concourse.tile as tile
    from concourse import bacc, mybir
    from concourse.masks import make_identity

    f32 = mybir.dt.float32
    f16 = mybir.dt.float16
    bf16 = mybir.dt.bfloat16
    act = mybir.ActivationFunctionType
    alu = mybir.AluOpType

    nc = bacc.Bacc("TRN2", target_bir_lowering=False, debug=False)
    # x fp16, per-tile partition-major blocks concatenated flat:
    # tile t occupies [128, NK, bt] at element offset 128*NK*OFFS[t], so
    # every group DMA is one contiguous gsz*bt*2-byte run per partition.
    xt = nc.dram_tensor("xt", [NK * 128 * BC], f16, kind="ExternalInput").ap()
    wt0 = nc.dram_tensor("wt0", [128, 8 * 128], f16, kind="ExternalInput").ap()
    wt1 = nc.dram_tensor("wt1", [128, (NK - 8) * 128], f16,
                         kind="ExternalInput").ap()
    bb = nc.dram_tensor("bb", [128, 2], f32, kind="ExternalInput").ap()
    out = nc.dram_tensor("out", [BC, 1], f32, kind="ExternalOutput").ap()

    with tile.TileContext(nc) as tc:
        with (
            tc.tile_pool(name="consts", bufs=1) as consts,
            tc.tile_pool(name="xpool", bufs=6) as xpool,
            tc.tile_pool(name="eppool", bufs=4) as eppool,
            tc.tile_pool(name="small", bufs=2) as small,
            tc.tile_pool(name="tvp", bufs=8) as tvp,
            tc.tile_pool(name="psacc", bufs=1, space=bass.MemorySpace.PSUM) as psacc,
            tc.tile_pool(name="pstr", bufs=2, space=bass.MemorySpace.PSUM) as pstr,
        ):
            # ---- constants ----
            # weights go out on the ACT sequencer: it reaches its first
            # instruction ~500ns before SP clears the Tile preamble, and is
            # otherwise idle until the first epilogue (~17us in)
            w0_sb = consts.tile([128, 8, 128], f16)
            nc.scalar.dma_start(out=w0_sb,
                                in_=wt0.rearrange("p (g m) -> p g m", g=8))
            w1_sb = consts.tile([128, NK - 8, 128], f16)
            nc.scalar.dma_start(out=w1_sb,
                                in_=wt1.rearrange("p (g m) -> p g m", g=NK - 8))
            bb_sb = consts.tile([128, 2], f32)
            nc.gpsimd.dma_start(out=bb_sb, in_=bb)
            ident = consts.tile([128, 128], f32)
            make_identity(nc, ident)
            # warm the ACT exp_and_others table during the DMA phase; every
            # later ACT op (Identity/Copy/Exp) stays in this one set.
            warm = consts.tile([1, 1], f32)
            nc.vector.memset(warm, 0.0)
            nc.scalar.add(warm, warm, bb_sb[0:1, 0:1])
            nc.scalar.activation(warm, warm, func=act.Exp)
            # tiles 0-3 stage their results here; one deferred DMA ships
            # them after the last x byte so no output transfer steals
            # mid-stream DMA time
            final_sb = consts.tile([128, 14], f32)

            accs = [psacc.tile([128, 512], f32, tag=f"acc{t}", name=f"acc{t}")
                    for t in range(len(TILES))]

            for t, bt in enumerate(TILES):
                njs = bt // 128
                off = OFFS[t]
                acc = accs[t][:, 0:bt]
                # ---- stream tile t's contraction, accumulate logits.T ----
                # acc[0:64,:] = noise logits.T, acc[64:128,:] = expert
                # logits.T (both pre-bias)
                base = 128 * NK * off
                xtile = xt[base:base + 128 * NK * bt].rearrange(
                    "(p k b) -> p k b", p=128, k=NK)
                k0 = 0
                for gsz in GROUPS[t]:
                    xk = xpool.tile([128, gsz, bt], f16, tag=f"xk{bt}_{gsz}")
                    nc.sync.dma_start(out=xk, in_=xtile[:, k0:k0 + gsz, :])
                    for g in range(gsz):
                        k = k0 + g
                        w = w0_sb[:, k, :] if k < 8 else w1_sb[:, k - 8, :]
                        nc.tensor.matmul(acc, lhsT=w, rhs=xk[:, g, :],
                                         start=(k == 0), stop=(k == NK - 1))
                    k0 += gsz

                # ---- epilogue for tile t (overlaps tile t+1's stream) ----
                # bias-add both halves PSUM->SBUF: noise on ACT, expert on
                # DVE, in parallel
                noiseT = eppool.tile([64, bt], f32, tag=f"nT{bt}")
                nc.scalar.add(noiseT, accs[t][0:64, 0:bt], bb_sb[0:64, 0:1])
                last = t == len(TILES) - 1
                if last:
                    # sigmoid built pre-transpose straight from PSUM:
                    # exp(-(z+b)) via scale=-1 and the staged -expert_b
                    eex64 = eppool.tile([64, bt], f32, tag="ee64")
                    nc.scalar.activation(eex64, accs[t][64:128, 0:bt],
                                         func=act.Exp, scale=-1.0,
                                         bias=bb_sb[64:128, 1:2])
                    den64 = eppool.tile([64, bt], f32, tag="de64")
                    nc.vector.tensor_scalar_add(den64, eex64, 1.0)
                    sig64 = eppool.tile([64, bt], f32, tag="sg64")
                    nc.vector.reciprocal(sig64, den64)
                else:
                    expT = eppool.tile([64, bt], f32, tag=f"eT{bt}")
                    nc.scalar.add(expT, accs[t][64:128, 0:bt],
                                  bb_sb[64:128, 0:1])
                # transpose to batch-major: [128 batch, j | 4+j, 64];
                # noise half first so e_all starts as early as possible
                ps_ne = pstr.tile([128, 8, 64], f32, tag="ps_ne",
                                  name=f"ps_ne{t}")
                for j in range(njs):
                    nc.tensor.transpose(ps_ne[:, j, :],
                                        noiseT[:, j * 128:(j + 1) * 128],
                                        ident[0:64, 0:64])
                if last:
                    # finished sigmoid lands in dead PSUM columns of the
                    # previous tile's (long-retired) accumulator bank, so
                    # ps_ne keeps ACT as its only reader
                    ps_sig = accs[t - 1][:, 384:448]
                    nc.tensor.transpose(ps_sig, sig64, ident[0:64, 0:64])
                else:
                    for j in range(njs):
                        nc.tensor.transpose(ps_ne[:, 4 + j, :],
                                            expT[:, j * 128:(j + 1) * 128],
                                            ident[0:64, 0:64])
                # softmax numerator without max-shift (|logit| <~ 4); the
                # only readers of ps_ne are the two ACT exps, so the DVE
                # chain below runs entirely from SBUF
                e_all = small.tile([128, 4, 64], f32, tag="e_all")
                nc.scalar.activation(e_all[:, 0:njs, :], ps_ne[:, 0:njs, :],
                                     func=act.Exp)
                if not last:
                    eex = small.tile([128, 4, 64], f32, tag="eex")
                    nc.scalar.activation(eex[:, 0:njs, :],
                                         ps_ne[:, 4:4 + njs, :],
                                         func=act.Exp, scale=-1.0)
                # top-8 on exp(v) (monotone => same selection as on v)
                tvs = []
                for j in range(njs):
                    tv = tvp.tile([128, 8], f32, tag="tv", name=f"tv{t}_{j}")
                    nc.vector.max(tv, e_all[:, j, :])
                    tvs.append(tv)
                # g = e where e >= e8 else 0; zsum = row sum of g
                gts = small.tile([128, 4, 64], f32, tag="gts")
                zsum = small.tile([128, 4], f32, tag="zsum")
                for j in range(njs):
                    nc.vector.scalar_tensor_tensor(
                        out=gts[:, j, :], in0=e_all[:, j, :],
                        scalar=tvs[j][:, 7:8], in1=e_all[:, j, :],
                        op0=alu.is_ge, op1=alu.mult,
                        accum_out=zsum[:, j:j + 1])
                if not last:
                    den = small.tile([128, 4, 64], f32, tag="den")
                    nc.vector.tensor_scalar_add(den[:, 0:njs, :],
                                                eex[:, 0:njs, :], 1.0)
                    sig = small.tile([128, 4, 64], f32, tag="sig")
                    nc.vector.reciprocal(sig[:, 0:njs, :], den[:, 0:njs, :])
                # s4 = sum_e g*sigmoid
                scr = small.tile([128, 4, 64], f32, tag="scr")
                s4 = small.tile([128, 4], f32, tag="s4")
                for j in range(njs):
                    sig_j = ps_sig if last else sig[:, j, :]
                    nc.vector.scalar_tensor_tensor(
                        out=scr[:, j, :], in0=gts[:, j, :], scalar=1.0,
                        in1=sig_j, op0=alu.mult, op1=alu.mult,
                        accum_out=s4[:, j:j + 1])
                rz = small.tile([128, 4], f32, tag="rz")
                nc.vector.reciprocal(rz[:, 0:njs], zsum[:, 0:njs])
                if t <= 3:
                    c0 = off // 128
                    nc.vector.tensor_mul(final_sb[:, c0:c0 + njs],
                                         s4[:, 0:njs], rz[:, 0:njs])
                    if t == 3:
                        nc.gpsimd.dma_start(
                            out=out[0:1792, :].rearrange(
                                "(j p) o -> p (j o)", j=14, p=128),
                            in_=final_sb)
                else:
                    fin = small.tile([128, 4], f32, tag="fin")
                    nc.vector.tensor_mul(fin[:, 0:njs], s4[:, 0:njs],
                                         rz[:, 0:njs])
                    out_t = out[off:off + bt, :].rearrange(
                        "(j p) o -> p (j o)", j=njs, p=128)
                    eng = nc.sync if t == len(TILES) - 1 else nc.gpsimd
                    eng.dma_start(out=out_t, in_=fin[:, 0:njs])

    nc.compile()
    return nc


def get_program():
    if "prog" not in _cached:
        _cached["prog"] = _build_program()
    return _cached["prog"]


def make_in_maps(x, noise_w, noise_b, expert_w, expert_b):
    """Host-side sharding: per-core transposed fp16 x slice + weights."""
    w_comb = np.concatenate([noise_w, expert_w], axis=0).astype(np.float32)  # [128, D]
    wt32 = np.ascontiguousarray(w_comb.T).astype(np.float16)                 # [D, 128]
    # partition p holds [nk, 128] for contraction rows nk*128+p
    wt = np.ascontiguousarray(
        wt32.reshape(NK, 128, 128).transpose(1, 0, 2).reshape(128, -1))
    wt0 = np.ascontiguousarray(wt[:, :8 * 128])
    wt1 = np.ascontiguousarray(wt[:, 8 * 128:])
    bb = np.stack([np.concatenate([noise_b, expert_b]),
                   np.concatenate([0 * noise_b, -expert_b])],
                  axis=1).astype(np.float32)
    in_maps = []
    for c in range(NCORES):
        xs = np.ascontiguousarray(x[c * BC:(c + 1) * BC, :].T).astype(np.float16)
        # per tile: [D, bt] -> [128, NK, bt], concatenated flat
        blocks = []
        for t, bt in enumerate(TILES):
            blk = xs[:, OFFS[t]:OFFS[t] + bt].reshape(NK, 128, bt)
            blocks.append(blk.transpose(1, 0, 2).reshape(-1))
        xr = np.ascontiguousarray(np.concatenate(blocks))
        in_maps.append({"xt": xr, "wt0": wt0, "wt1": wt1, "bb": bb})
    return in_maps


def kernel(x, noise, router_w, router_b, noise_w, noise_b, expert_w, expert_b,
           _trace=False):
    from concourse.bass_utils import run_bass_kernel_spmd

    x = np.asarray(x, dtype=np.float32)
    nc = get_program()
    in_maps = make_in_maps(x, np.asarray(noise_w), np.asarray(noise_b),
                           np.asarray(expert_w), np.asarray(expert_b))
    res = run_bass_kernel_spmd(nc, in_maps, core_ids=list(range(NCORES)),
                               trace=_trace)
    out = np.concatenate([r["out"] for r in res.results], axis=0)
    if _trace:
        kernel.last_results = res
    return out



# revision 5
# speedup vs baseline: 1.2875x; 1.2875x over previous
"""MoE logistic regression kernel for 8 Trainium2 NeuronCores.

Math (after dead-code elimination of the reference's unused router path):
    noise_logits = x @ noise_w.T + noise_b            # [B, E]
    top8 = top_k(noise_logits, 8)
    gates = softmax over the top-8 entries (others 0)
    expert = sigmoid(x @ expert_w.T + expert_b)       # [B, E]
    out[b] = sum_e gates[b,e] * expert[b,e]           # [B, 1]

Sharding: batch split 8 ways (2048 rows/core); weights replicated.

Implementation notes:
- Single-pass fp16 matmul (x, w rounded on host). Logit error ~2.3e-4
  flips the 8th/9th expert on ~24/16384 rows; end-to-end l2 rel err
  ~1.2e-3 vs the 2e-2 gate, at half the DMA and a third of the PE work
  of an fp16 hi/lo split. The x stream is the roofline: ~46.6us of
  gapless DMA at the HBM limit.
- Batch-tile-major stream: each tile's full contraction arrives while
  the previous tile's epilogue runs on ACT/DVE. Tile widths taper
  (512,512,512,256,128,128) and the last tile's DMA groups taper too,
  so the serial tail after the last byte is one short epilogue.
- The epilogue never uses the ACT sigmoid table: sigmoid comes from
  exp(-z) + DVE 1/(1+e), and softmax skips the max-shift (logits are
  bounded ~|4|), so every ACT op stays in the one exp_and_others
  function set -- no mid-stream LoadActFuncSet (1.3us each).
- Top-8 gating via DVE Max8 + fused (e >= e8) mask * e with accumulated
  row sum (scalar_tensor_tensor), all on the SBUF exp(v) copy --
  exp is monotone so selection is identical, and avoiding a second
  PSUM reader dodges cross-engine read serialization.
- Per-tile outputs DMA straight from the [128, j] result (no final
  transpose); early tiles go via the idle gpsimd SWDGE path so they
  never head-of-line-block the x stream on the SP HWDGE queue.
"""

import sys

import numpy as np

if "/opt/trn_rl_repo" not in sys.path:
    sys.path.insert(0, "/opt/trn_rl_repo")

B, D, E, TOPK, NCORES = 16384, 4096, 64, 8, 8
BC = B // NCORES      # batch rows per core
NK = D // 128         # contraction chunks
TILES = [512, 512, 512, 256, 128, 128]          # batch tile widths
OFFS = [sum(TILES[:i]) for i in range(len(TILES))]
assert sum(TILES) == BC
# DMA grouping in k-chunks per tile; the final tile tapers so almost no
# matmul work remains after the last byte lands
GROUPS = [[8, 8, 8, 8]] * 5 + [[16, 8, 4, 2, 2]]

_cached = {}


def _build_program():
    import concourse.bass as bass
    import concourse.tile as tile
    from concourse import bacc, mybir
    from concourse.masks import make_identity

    f32 = mybir.dt.float32
    f16 = mybir.dt.float16
    bf16 = mybir.dt.bfloat16
    f8e3 = mybir.dt.float8e3
    act = mybir.ActivationFunctionType
    alu = mybir.AluOpType

    nc = bacc.Bacc("TRN2", target_bir_lowering=False, debug=False)
    # x fp8 (e3m4), per-tile partition-major blocks concatenated flat:
    # tile t occupies [128, NK, bt] at element offset 128*NK*OFFS[t], so
    # every group DMA is one contiguous gsz*bt-byte run per partition.
    xt = nc.dram_tensor("xt", [NK * 128 * BC], f8e3, kind="ExternalInput").ap()
    wt0 = nc.dram_tensor("wt0", [128, 8 * 128], f16, kind="ExternalInput").ap()
    wt1 = nc.dram_tensor("wt1", [128, (NK - 8) * 128], f16,
                         kind="ExternalInput").ap()
    bb = nc.dram_tensor("bb", [128, 2], f32, kind="ExternalInput").ap()
    out = nc.dram_tensor("out", [BC, 1], f32, kind="ExternalOutput").ap()

    with tile.TileContext(nc) as tc:
        with (
            tc.tile_pool(name="consts", bufs=1) as consts,
            tc.tile_pool(name="xpool", bufs=6) as xpool,
            tc.tile_pool(name="eppool", bufs=4) as eppool,
            tc.tile_pool(name="small", bufs=2) as small,
            tc.tile_pool(name="tvp", bufs=8) as tvp,
            tc.tile_pool(name="psacc", bufs=1, space=bass.MemorySpace.PSUM) as psacc,
            tc.tile_pool(name="pstr", bufs=2, space=bass.MemorySpace.PSUM) as pstr,
        ):
            # ---- constants ----
            # weights go out on the ACT sequencer: it reaches its first
            # instruction ~500ns before SP clears the Tile preamble, and is
            # otherwise idle until the first epilogue (~17us in)
            w0_sb = consts.tile([128, 8, 128], f16)
            nc.scalar.dma_start(out=w0_sb,
                                in_=wt0.rearrange("p (g m) -> p g m", g=8))
            w1_sb = consts.tile([128, NK - 8, 128], f16)
            nc.scalar.dma_start(out=w1_sb,
                                in_=wt1.rearrange("p (g m) -> p g m", g=NK - 8))
            bb_sb = consts.tile([128, 2], f32)
            nc.gpsimd.dma_start(out=bb_sb, in_=bb)
            ident = consts.tile([128, 128], f32)
            make_identity(nc, ident)
            # warm the ACT exp_and_others table during the DMA phase; every
            # later ACT op (Identity/Copy/Exp) stays in this one set.
            warm = consts.tile([1, 1], f32)
            nc.vector.memset(warm, 0.0)
            nc.scalar.add(warm, warm, bb_sb[0:1, 0:1])
            nc.scalar.activation(warm, warm, func=act.Exp)
            # tiles 0-3 stage their results here; one deferred DMA ships
            # them after the last x byte so no output transfer steals
            # mid-stream DMA time
            final_sb = consts.tile([128, 14], f32)

            accs = [psacc.tile([128, 512], f32, tag=f"acc{t}", name=f"acc{t}")
                    for t in range(len(TILES))]

            for t, bt in enumerate(TILES):
                njs = bt // 128
                off = OFFS[t]
                acc = accs[t][:, 0:bt]
                # ---- stream tile t's contraction, accumulate logits.T ----
                # acc[0:64,:] = noise logits.T, acc[64:128,:] = expert
                # logits.T (both pre-bias)
                base = 128 * NK * off
                xtile = xt[base:base + 128 * NK * bt].rearrange(
                    "(p k b) -> p k b", p=128, k=NK)
                k0 = 0
                for gsz in GROUPS[t]:
                    xk = xpool.tile([128, gsz, bt], f8e3, tag=f"xk{bt}_{gsz}")
                    nc.sync.dma_start(out=xk, in_=xtile[:, k0:k0 + gsz, :])
                    for g in range(gsz):
                        k = k0 + g
                        w = w0_sb[:, k, :] if k < 8 else w1_sb[:, k - 8, :]
                        nc.tensor.matmul(acc, lhsT=w, rhs=xk[:, g, :],
                                         start=(k == 0), stop=(k == NK - 1))
                    k0 += gsz

                # ---- epilogue for tile t (overlaps tile t+1's stream) ----
                # bias-add both halves PSUM->SBUF: noise on ACT, expert on
                # DVE, in parallel
                noiseT = eppool.tile([64, bt], f32, tag=f"nT{bt}")
                nc.scalar.add(noiseT, accs[t][0:64, 0:bt], bb_sb[0:64, 0:1])
                last = t == len(TILES) - 1
                if last:
                    # sigmoid built pre-transpose straight from PSUM:
                    # exp(-(z+b)) via scale=-1 and the staged -expert_b
                    eex64 = eppool.tile([64, bt], f32, tag="ee64")
                    nc.scalar.activation(eex64, accs[t][64:128, 0:bt],
                                         func=act.Exp, scale=-1.0,
                                         bias=bb_sb[64:128, 1:2])
                    den64 = eppool.tile([64, bt], f32, tag="de64")
                    nc.vector.tensor_scalar_add(den64, eex64, 1.0)
                    sig64 = eppool.tile([64, bt], f32, tag="sg64")
                    nc.vector.reciprocal(sig64, den64)
                else:
                    expT = eppool.tile([64, bt], f32, tag=f"eT{bt}")
                    nc.scalar.add(expT, accs[t][64:128, 0:bt],
                                  bb_sb[64:128, 0:1])
                # transpose to batch-major: [128 batch, j | 4+j, 64];
                # noise half first so e_all starts as early as possible
                ps_ne = pstr.tile([128, 8, 64], f32, tag="ps_ne",
                                  name=f"ps_ne{t}")
                for j in range(njs):
                    nc.tensor.transpose(ps_ne[:, j, :],
                                        noiseT[:, j * 128:(j + 1) * 128],
                                        ident[0:64, 0:64])
                if last:
                    # finished sigmoid lands in dead PSUM columns of the
                    # previous tile's (long-retired) accumulator bank, so
                    # ps_ne keeps ACT as its only reader
                    ps_sig = accs[t - 1][:, 384:448]
                    nc.tensor.transpose(ps_sig, sig64, ident[0:64, 0:64])
                else:
                    for j in range(njs):
                        nc.tensor.transpose(ps_ne[:, 4 + j, :],
                                            expT[:, j * 128:(j + 1) * 128],
                                            ident[0:64, 0:64])
                # softmax numerator without max-shift (|logit| <~ 4); the
                # only readers of ps_ne are the two ACT exps, so the DVE
                # chain below runs entirely from SBUF
                e_all = small.tile([128, 4, 64], f32, tag="e_all")
                nc.scalar.activation(e_all[:, 0:njs, :], ps_ne[:, 0:njs, :],
                                     func=act.Exp)
                if not last:
                    eex = small.tile([128, 4, 64], f32, tag="eex")
                    nc.scalar.activation(eex[:, 0:njs, :],
                                         ps_ne[:, 4:4 + njs, :],
                                         func=act.Exp, scale=-1.0)
                # top-8 on exp(v) (monotone => same selection as on v)
                tvs = []
                for j in range(njs):
                    tv = tvp.tile([128, 8], f32, tag="tv", name=f"tv{t}_{j}")
                    nc.vector.max(tv, e_all[:, j, :])
                    tvs.append(tv)
                # g = e where e >= e8 else 0; zsum = row sum of g
                gts = small.tile([128, 4, 64], f32, tag="gts")
                zsum = small.tile([128, 4], f32, tag="zsum")
                for j in range(njs):
                    nc.vector.scalar_tensor_tensor(
                        out=gts[:, j, :], in0=e_all[:, j, :],
                        scalar=tvs[j][:, 7:8], in1=e_all[:, j, :],
                        op0=alu.is_ge, op1=alu.mult,
                        accum_out=zsum[:, j:j + 1])
                if not last:
                    den = small.tile([128, 4, 64], f32, tag="den")
                    nc.vector.tensor_scalar_add(den[:, 0:njs, :],
                                                eex[:, 0:njs, :], 1.0)
                    sig = small.tile([128, 4, 64], f32, tag="sig")
                    nc.vector.reciprocal(sig[:, 0:njs, :], den[:, 0:njs, :])
                # s4 = sum_e g*sigmoid
                scr = small.tile([128, 4, 64], f32, tag="scr")
                s4 = small.tile([128, 4], f32, tag="s4")
                for j in range(njs):
                    sig_j = ps_sig if last else sig[:, j, :]
                    nc.vector.scalar_tensor_tensor(
                        out=scr[:, j, :], in0=gts[:, j, :], scalar=1.0,
                        in1=sig_j, op0=alu.mult, op1=alu.mult,
                        accum_out=s4[:, j:j + 1])
                rz = small.tile([128, 4], f32, tag="rz")
                nc.vector.reciprocal(rz[:, 0:njs], zsum[:, 0:njs])
                if t <= 3:
                    c0 = off // 128
                    nc.vector.tensor_mul(final_sb[:, c0:c0 + njs],
                                         s4[:, 0:njs], rz[:, 0:njs])
                    if t == 3:
                        nc.gpsimd.dma_start(
                            out=out[0:1792, :].rearrange(
                                "(j p) o -> p (j o)", j=14, p=128),
                            in_=final_sb)
                else:
                    fin = small.tile([128, 4], f32, tag="fin")
                    nc.vector.tensor_mul(fin[:, 0:njs], s4[:, 0:njs],
                                         rz[:, 0:njs])
                    out_t = out[off:off + bt, :].rearrange(
                        "(j p) o -> p (j o)", j=njs, p=128)
                    eng = nc.sync if t == len(TILES) - 1 else nc.gpsimd
                    eng.dma_start(out=out_t, in_=fin[:, 0:njs])

    nc.compile()
    return nc


def get_program():
    if "prog" not in _cached:
        _cached["prog"] = _build_program()
    return _cached["prog"]


def make_in_maps(x, noise_w, noise_b, expert_w, expert_b):
    """Host-side sharding: per-core transposed fp8(e3m4) x slice + weights."""
    import ml_dtypes
    w_comb = np.concatenate([noise_w, expert_w], axis=0).astype(np.float32)  # [128, D]
    wt32 = np.ascontiguousarray(w_comb.T).astype(np.float16)                 # [D, 128]
    # partition p holds [nk, 128] for contraction rows nk*128+p
    wt = np.ascontiguousarray(
        wt32.reshape(NK, 128, 128).transpose(1, 0, 2).reshape(128, -1))
    wt0 = np.ascontiguousarray(wt[:, :8 * 128])
    wt1 = np.ascontiguousarray(wt[:, 8 * 128:])
    bb = np.stack([np.concatenate([noise_b, expert_b]),
                   np.concatenate([0 * noise_b, -expert_b])],
                  axis=1).astype(np.float32)
    in_maps = []
    for c in range(NCORES):
        xs = np.ascontiguousarray(x[c * BC:(c + 1) * BC, :].T).astype(
            ml_dtypes.float8_e3m4)
        # per tile: [D, bt] -> [128, NK, bt], concatenated flat
        blocks = []
        for t, bt in enumerate(TILES):
            blk = xs[:, OFFS[t]:OFFS[t] + bt].reshape(NK, 128, bt)
            blocks.append(blk.transpose(1, 0, 2).reshape(-1))
        xr = np.ascontiguousarray(np.concatenate(blocks))
        in_maps.append({"xt": xr, "wt0": wt0, "wt1": wt1, "bb": bb})
    return in_maps


def kernel(x, noise, router_w, router_b, noise_w, noise_b, expert_w, expert_b,
           _trace=False):
    from concourse.bass_utils import run_bass_kernel_spmd

    x = np.asarray(x, dtype=np.float32)
    nc = get_program()
    in_maps = make_in_maps(x, np.asarray(noise_w), np.asarray(noise_b),
                           np.asarray(expert_w), np.asarray(expert_b))
    res = run_bass_kernel_spmd(nc, in_maps, core_ids=list(range(NCORES)),
                               trace=_trace)
    out = np.concatenate([r["out"] for r in res.results], axis=0)
    if _trace:
        kernel.last_results = res
    return out



# revision 8
# speedup vs baseline: 1.3822x; 1.0735x over previous
"""MoE logistic regression kernel for 8 Trainium2 NeuronCores.

Math (after dead-code elimination of the reference's unused router path):
    noise_logits = x @ noise_w.T + noise_b            # [B, E]
    top8 = top_k(noise_logits, 8)
    gates = softmax over the top-8 entries (others 0)
    expert = sigmoid(x @ expert_w.T + expert_b)       # [B, E]
    out[b] = sum_e gates[b,e] * expert[b,e]           # [B, 1]

Sharding: batch split 8 ways (2048 rows/core); weights replicated.

Implementation notes:
- Single-pass fp16 matmul (x, w rounded on host). Logit error ~2.3e-4
  flips the 8th/9th expert on ~24/16384 rows; end-to-end l2 rel err
  ~1.2e-3 vs the 2e-2 gate, at half the DMA and a third of the PE work
  of an fp16 hi/lo split. The x stream is the roofline: ~46.6us of
  gapless DMA at the HBM limit.
- Batch-tile-major stream: each tile's full contraction arrives while
  the previous tile's epilogue runs on ACT/DVE. Tile widths taper
  (512,512,512,256,128,128) and the last tile's DMA groups taper too,
  so the serial tail after the last byte is one short epilogue.
- The epilogue never uses the ACT sigmoid table: sigmoid comes from
  exp(-z) + DVE 1/(1+e), and softmax skips the max-shift (logits are
  bounded ~|4|), so every ACT op stays in the one exp_and_others
  function set -- no mid-stream LoadActFuncSet (1.3us each).
- Top-8 gating via DVE Max8 + fused (e >= e8) mask * e with accumulated
  row sum (scalar_tensor_tensor), all on the SBUF exp(v) copy --
  exp is monotone so selection is identical, and avoiding a second
  PSUM reader dodges cross-engine read serialization.
- Per-tile outputs DMA straight from the [128, j] result (no final
  transpose); early tiles go via the idle gpsimd SWDGE path so they
  never head-of-line-block the x stream on the SP HWDGE queue.
"""

import sys

import numpy as np

if "/opt/trn_rl_repo" not in sys.path:
    sys.path.insert(0, "/opt/trn_rl_repo")

B, D, E, TOPK, NCORES = 16384, 4096, 64, 8, 8
BC = B // NCORES      # batch rows per core
NK = D // 128         # contraction chunks
TILES = [512, 512, 512, 256, 128, 128]          # batch tile widths
OFFS = [sum(TILES[:i]) for i in range(len(TILES))]
assert sum(TILES) == BC
# DMA grouping in k-chunks per tile; the final tile tapers so almost no
# matmul work remains after the last byte lands. Tile 0 leads with a
# 2-chunk group so the first real matmul starts as early as possible.
GROUPS = [[2, 6, 8, 8, 8]] + [[8, 8, 8, 8]] * 4 + [[16, 8, 4, 2, 2]]

_cached = {}


def _build_program():
    import concourse.bass as bass
    import concourse.tile as tile
    from concourse import bacc, mybir
    from concourse.masks import make_identity

    f32 = mybir.dt.float32
    f16 = mybir.dt.float16
    bf16 = mybir.dt.bfloat16
    f8e3 = mybir.dt.float8e3
    act = mybir.ActivationFunctionType
    alu = mybir.AluOpType

    nc = bacc.Bacc("TRN2", target_bir_lowering=False, debug=False)
    # x fp8 (e3m4), per-tile partition-major blocks concatenated flat:
    # tile t occupies [128, NK, bt] at element offset 128*NK*OFFS[t], so
    # every group DMA is one contiguous gsz*bt-byte run per partition.
    xt = nc.dram_tensor("xt", [NK * 128 * BC], f8e3, kind="ExternalInput").ap()
    wt0 = nc.dram_tensor("wt0", [128, 8 * 128], f16, kind="ExternalInput").ap()
    wt1 = nc.dram_tensor("wt1", [128, (NK - 8) * 128], f16,
                         kind="ExternalInput").ap()
    bb = nc.dram_tensor("bb", [128, 2], f32, kind="ExternalInput").ap()
    out = nc.dram_tensor("out", [BC, 1], f32, kind="ExternalOutput").ap()

    with tile.TileContext(nc) as tc:
        with (
            tc.tile_pool(name="consts", bufs=1) as consts,
            tc.tile_pool(name="xpool", bufs=6) as xpool,
            tc.tile_pool(name="eppool", bufs=4) as eppool,
            tc.tile_pool(name="small", bufs=2) as small,
            tc.tile_pool(name="tvp", bufs=8) as tvp,
            tc.tile_pool(name="psacc", bufs=1, space=bass.MemorySpace.PSUM) as psacc,
            tc.tile_pool(name="pstr", bufs=2, space=bass.MemorySpace.PSUM) as pstr,
        ):
            # ---- constants ----
            # w0 rides the gpsimd SWDGE queue (its generation starts ~1us
            # before the SP HWDGE path clears the Tile preamble); w1 is
            # split so neither piece blocks the x stream for long, and each
            # piece lands just before the matmuls that need it.
            w0_sb = consts.tile([128, 8, 128], f16)
            nc.gpsimd.dma_start(out=w0_sb,
                                in_=wt0.rearrange("p (g m) -> p g m", g=8))
            w1_sb = consts.tile([128, NK - 8, 128], f16)
            nc.scalar.dma_start(
                out=w1_sb[:, 0:8, :],
                in_=wt1.rearrange("p (g m) -> p g m", g=NK - 8)[:, 0:8, :])
            nc.scalar.dma_start(
                out=w1_sb[:, 8:NK - 8, :],
                in_=wt1.rearrange("p (g m) -> p g m", g=NK - 8)[:, 8:NK - 8, :])
            bb_sb = consts.tile([128, 2], f32)
            nc.gpsimd.dma_start(out=bb_sb, in_=bb)
            ident = consts.tile([128, 128], f32)
            make_identity(nc, ident)
            # warm the ACT exp_and_others table during the DMA phase; every
            # later ACT op (Identity/Copy/Exp) stays in this one set.
            warm = consts.tile([1, 1], f32)
            nc.vector.memset(warm, 0.0)
            nc.scalar.add(warm, warm, bb_sb[0:1, 0:1])
            nc.scalar.activation(warm, warm, func=act.Exp)
            # tiles 0-3 stage their results here; one deferred DMA ships
            # them after the last x byte so no output transfer steals
            # mid-stream DMA time
            final_sb = consts.tile([128, 14], f32)

            accs = [psacc.tile([128, 512], f32, tag=f"acc{t}", name=f"acc{t}")
                    for t in range(len(TILES))]

            # PE p-state warm-up: the cost of a matmul drops 0.65->1.2->2.4
            # GHz only after ~3us of continuous PE execution. Burn that ramp
            # on dummy matmuls (ident -> acc5 scratch, no DMA deps) during
            # the head where PE would otherwise idle, so every real matmul
            # runs at full clock. acc5's first real matmul start=True resets
            # the bank.
            for wi in range(9):
                nc.tensor.matmul(accs[5][:, 0:128], lhsT=ident, rhs=ident,
                                 start=True, stop=True)

            for t, bt in enumerate(TILES):
                njs = bt // 128
                off = OFFS[t]
                acc = accs[t][:, 0:bt]
                # ---- stream tile t's contraction, accumulate logits.T ----
                # acc[0:64,:] = noise logits.T, acc[64:128,:] = expert
                # logits.T (both pre-bias)
                base = 128 * NK * off
                xtile = xt[base:base + 128 * NK * bt].rearrange(
                    "(p k b) -> p k b", p=128, k=NK)
                k0 = 0
                for gsz in GROUPS[t]:
                    xk = xpool.tile([128, gsz, bt], f8e3, tag=f"xk{bt}_{gsz}")
                    nc.sync.dma_start(out=xk, in_=xtile[:, k0:k0 + gsz, :])
                    for g in range(gsz):
                        k = k0 + g
                        w = w0_sb[:, k, :] if k < 8 else w1_sb[:, k - 8, :]
                        nc.tensor.matmul(acc, lhsT=w, rhs=xk[:, g, :],
                                         start=(k == 0), stop=(k == NK - 1))
                    k0 += gsz

                # ---- epilogue for tile t (overlaps tile t+1's stream) ----
                # bias-add both halves PSUM->SBUF: noise on ACT, expert on
                # DVE, in parallel
                noiseT = eppool.tile([64, bt], f32, tag=f"nT{bt}")
                nc.scalar.add(noiseT, accs[t][0:64, 0:bt], bb_sb[0:64, 0:1])
                last = t == len(TILES) - 1
                if last:
                    # sigmoid built pre-transpose straight from PSUM:
                    # exp(-(z+b)) via scale=-1 and the staged -expert_b
                    eex64 = eppool.tile([64, bt], f32, tag="ee64")
                    nc.scalar.activation(eex64, accs[t][64:128, 0:bt],
                                         func=act.Exp, scale=-1.0,
                                         bias=bb_sb[64:128, 1:2])
                    den64 = eppool.tile([64, bt], f32, tag="de64")
                    nc.vector.tensor_scalar_add(den64, eex64, 1.0)
                    sig64 = eppool.tile([64, bt], f32, tag="sg64")
                    nc.vector.reciprocal(sig64, den64)
                else:
                    expT = eppool.tile([64, bt], f32, tag=f"eT{bt}")
                    nc.scalar.add(expT, accs[t][64:128, 0:bt],
                                  bb_sb[64:128, 0:1])
                # transpose to batch-major: [128 batch, j | 4+j, 64];
                # noise half first so e_all starts as early as possible
                ps_ne = pstr.tile([128, 8, 64], f32, tag="ps_ne",
                                  name=f"ps_ne{t}")
                for j in range(njs):
                    nc.tensor.transpose(ps_ne[:, j, :],
                                        noiseT[:, j * 128:(j + 1) * 128],
                                        ident[0:64, 0:64])
                if last:
                    # finished sigmoid lands in dead PSUM columns of the
                    # previous tile's (long-retired) accumulator bank, so
                    # ps_ne keeps ACT as its only reader
                    ps_sig = accs[t - 1][:, 384:448]
                    nc.tensor.transpose(ps_sig, sig64, ident[0:64, 0:64])
                else:
                    for j in range(njs):
                        nc.tensor.transpose(ps_ne[:, 4 + j, :],
                                            expT[:, j * 128:(j + 1) * 128],
                                            ident[0:64, 0:64])
                # softmax numerator without max-shift (|logit| <~ 4); the
                # only readers of ps_ne are the two ACT exps, so the DVE
                # chain below runs entirely from SBUF
                e_all = small.tile([128, 4, 64], f32, tag="e_all")
                nc.scalar.activation(e_all[:, 0:njs, :], ps_ne[:, 0:njs, :],
                                     func=act.Exp)
                if not last:
                    eex = small.tile([128, 4, 64], f32, tag="eex")
                    nc.scalar.activation(eex[:, 0:njs, :],
                                         ps_ne[:, 4:4 + njs, :],
                                         func=act.Exp, scale=-1.0)
                # top-8 on exp(v) (monotone => same selection as on v)
                tvs = []
                for j in range(njs):
                    tv = tvp.tile([128, 8], f32, tag="tv", name=f"tv{t}_{j}")
                    nc.vector.max(tv, e_all[:, j, :])
                    tvs.append(tv)
                # g = e where e >= e8 else 0; zsum = row sum of g
                gts = small.tile([128, 4, 64], f32, tag="gts")
                zsum = small.tile([128, 4], f32, tag="zsum")
                for j in range(njs):
                    nc.vector.scalar_tensor_tensor(
                        out=gts[:, j, :], in0=e_all[:, j, :],
                        scalar=tvs[j][:, 7:8], in1=e_all[:, j, :],
                        op0=alu.is_ge, op1=alu.mult,
                        accum_out=zsum[:, j:j + 1])
                if not last:
                    den = small.tile([128, 4, 64], f32, tag="den")
                    nc.vector.tensor_scalar_add(den[:, 0:njs, :],
                                                eex[:, 0:njs, :], 1.0)
                    sig = small.tile([128, 4, 64], f32, tag="sig")
                    nc.vector.reciprocal(sig[:, 0:njs, :], den[:, 0:njs, :])
                # s4 = sum_e g*sigmoid
                scr = small.tile([128, 4, 64], f32, tag="scr")
                s4 = small.tile([128, 4], f32, tag="s4")
                for j in range(njs):
                    sig_j = ps_sig if last else sig[:, j, :]
                    nc.vector.scalar_tensor_tensor(
                        out=scr[:, j, :], in0=gts[:, j, :], scalar=1.0,
                        in1=sig_j, op0=alu.mult, op1=alu.mult,
                        accum_out=s4[:, j:j + 1])
                rz = small.tile([128, 4], f32, tag="rz")
                nc.vector.reciprocal(rz[:, 0:njs], zsum[:, 0:njs])
                if t <= 3:
                    c0 = off // 128
                    nc.vector.tensor_mul(final_sb[:, c0:c0 + njs],
                                         s4[:, 0:njs], rz[:, 0:njs])
                    if t == 3:
                        nc.gpsimd.dma_start(
                            out=out[0:1792, :].rearrange(
                                "(j p) o -> p (j o)", j=14, p=128),
                            in_=final_sb)
                else:
                    fin = small.tile([128, 4], f32, tag="fin")
                    nc.vector.tensor_mul(fin[:, 0:njs], s4[:, 0:njs],
                                         rz[:, 0:njs])
                    out_t = out[off:off + bt, :].rearrange(
                        "(j p) o -> p (j o)", j=njs, p=128)
                    eng = nc.sync if t == len(TILES) - 1 else nc.gpsimd
                    eng.dma_start(out=out_t, in_=fin[:, 0:njs])

    nc.compile()
    return nc


def get_program():
    if "prog" not in _cached:
        _cached["prog"] = _build_program()
    return _cached["prog"]


def make_in_maps(x, noise_w, noise_b, expert_w, expert_b):
    """Host-side sharding: per-core transposed fp8(e3m4) x slice + weights."""
    import ml_dtypes
    w_comb = np.concatenate([noise_w, expert_w], axis=0).astype(np.float32)  # [128, D]
    wt32 = np.ascontiguousarray(w_comb.T).astype(np.float16)                 # [D, 128]
    # partition p holds [nk, 128] for contraction rows nk*128+p
    wt = np.ascontiguousarray(
        wt32.reshape(NK, 128, 128).transpose(1, 0, 2).reshape(128, -1))
    wt0 = np.ascontiguousarray(wt[:, :8 * 128])
    wt1 = np.ascontiguousarray(wt[:, 8 * 128:])
    bb = np.stack([np.concatenate([noise_b, expert_b]),
                   np.concatenate([0 * noise_b, -expert_b])],
                  axis=1).astype(np.float32)
    in_maps = []
    for c in range(NCORES):
        xs = np.ascontiguousarray(x[c * BC:(c + 1) * BC, :].T).astype(
            ml_dtypes.float8_e3m4)
        # per tile: [D, bt] -> [128, NK, bt], concatenated flat
        blocks = []
        for t, bt in enumerate(TILES):
            blk = xs[:, OFFS[t]:OFFS[t] + bt].reshape(NK, 128, bt)
            blocks.append(blk.transpose(1, 0, 2).reshape(-1))
        xr = np.ascontiguousarray(np.concatenate(blocks))
        in_maps.append({"xt": xr, "wt0": wt0, "wt1": wt1, "bb": bb})
    return in_maps


def kernel(x, noise, router_w, router_b, noise_w, noise_b, expert_w, expert_b,
           _trace=False):
    from concourse.bass_utils import run_bass_kernel_spmd

    x = np.asarray(x, dtype=np.float32)
    nc = get_program()
    in_maps = make_in_maps(x, np.asarray(noise_w), np.asarray(noise_b),
                           np.asarray(expert_w), np.asarray(expert_b))
    res = run_bass_kernel_spmd(nc, in_maps, core_ids=list(range(NCORES)),
                               trace=_trace)
    out = np.concatenate([r["out"] for r in res.results], axis=0)
    if _trace:
        kernel.last_results = res
    return out



# revision 50
# speedup vs baseline: 1.4186x; 1.0264x over previous
"""MoE logistic regression kernel for 8 Trainium2 NeuronCores.

Math (after dead-code elimination of the reference's unused router path):
    noise_logits = x @ noise_w.T + noise_b            # [B, E]
    top8 = top_k(noise_logits, 8)
    gates = softmax over the top-8 entries (others 0)
    expert = sigmoid(x @ expert_w.T + expert_b)       # [B, E]
    out[b] = sum_e gates[b,e] * expert[b,e]           # [B, 1]

Sharding: batch split 8 ways (2048 rows/core); weights replicated.

Implementation notes:
- x streams in fp8 (e3m4), weights in fp8 (e3m4, x512 host prescale so the
  U(-1/64,1/64) weights land in e3m4's normal range; the 1/512 descale
  folds into the ACT activation scale for free). Halves both the DMA
  bytes and keeps the PE at 1 cycle/row. End-to-end l2 rel err ~1.3e-2
  vs the 2e-2 gate (top-8 selection flips on ~2k/16384 rows dominate).
- Batch-tile-major stream: each tile's full contraction arrives while
  the previous tile's epilogue runs on ACT/DVE. PE is the critical
  resource (65536 matmul cycles ~27.3us at 2.4GHz); the shared DMA pipe
  carries ~27us (x 25.3us + fp8 weights 1.6us).
- PE p-state warm-up: dummy matmuls burn the 0.65/1.2GHz ramp before the
  first real matmul so the real stream runs at 2.4GHz throughout.
- Fused epilogue: exp(+/-(z*s+b)) comes straight off PSUM on ACT (one op
  per half), transposes to batch-major on PE, and the whole gating chain
  runs on DVE: top-8 via Max8, zsum/s4 via two independent accumulating
  scalar_tensor_tensor selects (es = e*sigmoid precomputed), final
  s4/zsum. All ACT ops stay in the one exp_and_others function set.
- Per-tile outputs DMA straight from the [128, j] result; early tiles
  batch through final_sb on the idle gpsimd SWDGE path.
"""

import sys

import numpy as np

if "/opt/trn_rl_repo" not in sys.path:
    sys.path.insert(0, "/opt/trn_rl_repo")

B, D, E, TOPK, NCORES = 16384, 4096, 64, 8, 8
BC = B // NCORES      # batch rows per core
NK = D // 128         # contraction chunks
WSCALE = 512.0        # host weight prescale (descaled in the epilogue)
# Tile 0 is SMALL (256): the DMA pipe starts ~2us late and also carries
# the weights early on, so a narrow first tile halves the PE's early
# consumption rate and the pipe catches up within tile 0. Middle tiles
# are big so epilogue chains hide in their matmul windows; the last tile
# is small so its exposed chain is short.
TILES = [256, 512, 512, 512, 128, 128]          # batch tile widths
OFFS = [sum(TILES[:i]) for i in range(len(TILES))]
assert sum(TILES) == BC
# DMA grouping in k-chunks per tile; tile 0 leads with tiny groups so
# the first real matmul starts early, the final tile tapers so almost no
# matmul work remains after the last byte lands
GROUPS = [[2, 2, 4, 8, 8, 8]] + [[8, 8, 8, 8]] * 4 + [[16, 8, 4, 2, 2]]

_cached = {}


def _build_program():
    import concourse.bass as bass
    import concourse.tile as tile
    from concourse import bacc, mybir
    from concourse.masks import make_identity

    f32 = mybir.dt.float32
    f16 = mybir.dt.float16
    f8e3 = mybir.dt.float8e3
    act = mybir.ActivationFunctionType
    alu = mybir.AluOpType

    nc = bacc.Bacc("TRN2", target_bir_lowering=False, debug=False)
    # x fp8 (e3m4), per-tile partition-major blocks concatenated flat:
    # tile t occupies [128, NK, bt] at element offset 128*NK*OFFS[t], so
    # every group DMA is one contiguous gsz*bt-byte run per partition.
    xt = nc.dram_tensor("xt", [NK * 128 * BC], f8e3, kind="ExternalInput").ap()
    wt0 = nc.dram_tensor("wt0", [128, 8 * 128], f8e3, kind="ExternalInput").ap()
    wt1 = nc.dram_tensor("wt1", [128, (NK - 8) * 128], f8e3,
                         kind="ExternalInput").ap()
    bb = nc.dram_tensor("bb", [128, 2], f32, kind="ExternalInput").ap()
    out = nc.dram_tensor("out", [BC, 1], f32, kind="ExternalOutput").ap()

    with tile.TileContext(nc) as tc:
        with (
            tc.tile_pool(name="consts", bufs=1) as consts,
            tc.tile_pool(name="xpool", bufs=6) as xpool,
            tc.tile_pool(name="eppool", bufs=4) as eppool,
            tc.tile_pool(name="small", bufs=4) as small,
            tc.tile_pool(name="tvp", bufs=8) as tvp,
            tc.tile_pool(name="psacc", bufs=1, space=bass.MemorySpace.PSUM) as psacc,
            tc.tile_pool(name="pstr", bufs=2, space=bass.MemorySpace.PSUM) as pstr,
        ):
            # ---- constants ----
            accs = [psacc.tile([128, 512], f32, tag=f"acc{t}", name=f"acc{t}")
                    for t in range(len(TILES))]
            # identity FIRST on Pool: the PE warm-up dummies read it, so it
            # must not queue behind SWDGE descriptor generation.
            ident = consts.tile([128, 128], f32)
            make_identity(nc, ident)
            # PE p-state warm-up: matmul speed ramps 0.65->1.2->2.4 GHz only
            # after ~3us of continuous PE execution. Burn the ramp on dummy
            # 64-col matmuls (ident source, ready ~0.6us) so every real
            # matmul runs at full clock. The last acc's first real matmul
            # start=True resets the bank.
            for wi in range(14):
                nc.tensor.matmul(accs[-1][0:64, 0:64], lhsT=ident[:, 0:64],
                                 rhs=ident[:, 0:64], start=True, stop=True)
            # w0 leads the SP queue: it is the first dependency of the
            # first real matmul and SP's HWDGE pipe is the earliest to
            # start transfers (~1.3us in).
            w0_sb = consts.tile([128, 8, 128], f8e3)
            nc.sync.dma_start(out=w0_sb,
                              in_=wt0.rearrange("p (g m) -> p g m", g=8))
            bb_sb = consts.tile([128, 2], f32)
            nc.gpsimd.dma_start(out=bb_sb, in_=bb)
            # w1 split in three on ACT so each piece slots between x groups
            # on the shared transfer pipe, landing just before the matmuls
            # that need it.
            w1_sb = consts.tile([128, NK - 8, 128], f8e3)
            w1r = wt1.rearrange("p (g m) -> p g m", g=NK - 8)
            nc.scalar.dma_start(out=w1_sb[:, 0:8, :], in_=w1r[:, 0:8, :])
            nc.scalar.dma_start(out=w1_sb[:, 8:16, :], in_=w1r[:, 8:16, :])
            nc.scalar.dma_start(out=w1_sb[:, 16:24, :], in_=w1r[:, 16:24, :])
            # warm the ACT exp_and_others table during the DMA phase; every
            # later ACT op (Identity/Copy/Exp) stays in this one set.
            warm = consts.tile([1, 1], f32)
            nc.vector.memset(warm, 0.0)
            nc.scalar.add(warm, warm, bb_sb[0:1, 0:1])
            nc.scalar.activation(warm, warm, func=act.Exp)

            for t, bt in enumerate(TILES):
                njs = bt // 128
                off = OFFS[t]
                acc = accs[t][:, 0:bt]
                # ---- stream tile t's contraction, accumulate logits.T ----
                # acc[0:64,:] = WSCALE*noise logits.T, acc[64:128,:] =
                # WSCALE*expert logits.T (both pre-bias)
                base = 128 * NK * off
                xtile = xt[base:base + 128 * NK * bt].rearrange(
                    "(p k b) -> p k b", p=128, k=NK)
                k0 = 0
                for gsz in GROUPS[t]:
                    xk = xpool.tile([128, gsz, bt], f8e3, tag=f"xk{bt}_{gsz}")
                    nc.sync.dma_start(out=xk, in_=xtile[:, k0:k0 + gsz, :])
                    for g in range(gsz):
                        k = k0 + g
                        w = w0_sb[:, k, :] if k < 8 else w1_sb[:, k - 8, :]
                        nc.tensor.matmul(acc, lhsT=w, rhs=xk[:, g, :],
                                         start=(k == 0), stop=(k == NK - 1))
                    k0 += gsz

                # ---- epilogue for tile t (overlaps tile t+1's stream) ----
                # eA = exp(noise_logit + nb), eB = exp(-(expert_logit + eb)),
                # both straight off PSUM with the 1/WSCALE descale folded
                # in. eB FIRST: the longest chain runs through
                # eB->transpose->den->sig->es, so it must clear ACT first.
                eB = eppool.tile([64, bt], f32, tag=f"eB{bt}")
                nc.scalar.activation(eB, accs[t][64:128, 0:bt], func=act.Exp,
                                     scale=-1.0 / WSCALE,
                                     bias=bb_sb[64:128, 1:2])
                eA = eppool.tile([64, bt], f32, tag=f"eA{bt}")
                nc.scalar.activation(eA, accs[t][0:64, 0:bt], func=act.Exp,
                                     scale=1.0 / WSCALE,
                                     bias=bb_sb[0:64, 0:1])
                # transpose to batch-major: [128 batch, j | 4+j, 64];
                # expert half first (critical chain). The last tile stages
                # in the long-retired acc0 bank so it never waits on the
                # pstr rotation.
                if t < 4:
                    ps = pstr.tile([128, 8, 64], f32, tag="ps", name=f"ps{t}")
                    psA = [ps[:, j, :] for j in range(njs)]
                    psB = [ps[:, 4 + j, :] for j in range(njs)]
                    psBall = ps[:, 4:4 + njs, :]
                else:
                    psA = [accs[t - 4][:, j * 64:(j + 1) * 64]
                           for j in range(njs)]
                    psB = [accs[t - 4][:, 256 + j * 64:256 + (j + 1) * 64]
                           for j in range(njs)]
                    psBall = accs[t - 4][:, 256:256 + njs * 64]
                for j in range(njs):
                    nc.tensor.transpose(psB[j],
                                        eB[:, j * 128:(j + 1) * 128],
                                        ident[0:64, 0:64])
                for j in range(njs):
                    nc.tensor.transpose(psA[j],
                                        eA[:, j * 128:(j + 1) * 128],
                                        ident[0:64, 0:64])
                # den = 1 + eB; sigmoid = 1/den -- emitted before tv/zred so
                # the sig chain (which es and the final select depend on)
                # clears DVE first
                den = small.tile([128, 4, 64], f32, tag="den")
                if t < 4:
                    nc.vector.tensor_scalar_add(den[:, 0:njs, :], psBall, 1.0)
                else:
                    for j in range(njs):
                        nc.vector.tensor_scalar_add(den[:, j, :], psB[j], 1.0)
                sig = small.tile([128, 4, 64], f32, tag="sig")
                nc.vector.reciprocal(sig[:, 0:njs, :], den[:, 0:njs, :])
                # top-8 on exp(v) (monotone => same selection as on v);
                # zsum = sum of the top-8 values in ONE reduce over tv
                tv = tvp.tile([128, 32], f32, tag="tv", name=f"tv{t}")
                for j in range(njs):
                    nc.vector.max(tv[:, j * 8:(j + 1) * 8], psA[j])
                zsum = small.tile([128, 4], f32, tag="zsum")
                nc.vector.tensor_reduce(
                    zsum[:, 0:njs],
                    tv.rearrange("p (j k) -> p j k", k=8)[:, 0:njs, :],
                    axis=mybir.AxisListType.X, op=alu.add)
                # es = e * sigmoid. All ps reads stay on DVE so the PSUM
                # staging buffer frees as soon as the DVE chain drains.
                es = small.tile([128, 4, 64], f32, tag="es")
                for j in range(njs):
                    nc.vector.tensor_mul(es[:, j, :], psA[j], sig[:, j, :])
                # s4 = sum of top-8 e*sigmoid (accumulating select)
                s4 = small.tile([128, 4], f32, tag="s4")
                scr = small.tile([128, 4, 64], f32, tag="scr")
                for j in range(njs):
                    nc.vector.scalar_tensor_tensor(
                        out=scr[:, j, :], in0=psA[j],
                        scalar=tv[:, j * 8 + 7:j * 8 + 8], in1=es[:, j, :],
                        op0=alu.is_ge, op1=alu.mult,
                        accum_out=s4[:, j:j + 1])
                rz = small.tile([128, 4], f32, tag="rz")
                nc.vector.reciprocal(rz[:, 0:njs], zsum[:, 0:njs])
                fin = small.tile([128, 4], f32, tag="fin")
                nc.vector.tensor_mul(fin[:, 0:njs], s4[:, 0:njs],
                                     rz[:, 0:njs])
                # outputs ride the gpsimd SWDGE queue mid-stream (Pool is
                # idle once the weight gens finish; the ACT queue is
                # in-order, so an out issue there would block the next
                # tile's eA). The last tile uses SP, idle once the x stream
                # is issued.
                out_t = out[off:off + bt, :].rearrange(
                    "(j p) o -> p (j o)", j=njs, p=128)
                eng = nc.sync if t == len(TILES) - 1 else nc.gpsimd
                eng.dma_start(out=out_t, in_=fin[:, 0:njs])

    nc.compile()
    return nc


def get_program():
    if "prog" not in _cached:
        _cached["prog"] = _build_program()
    return _cached["prog"]


def make_in_maps(x, noise_w, noise_b, expert_w, expert_b):
    """Host-side sharding: per-core transposed fp8(e3m4) x + weights."""
    import ml_dtypes
    w_comb = np.concatenate([noise_w, expert_w], axis=0).astype(np.float32)
    wt32 = np.ascontiguousarray(w_comb.T) * np.float32(WSCALE)   # [D, 128]
    # partition p holds [nk, 128] for contraction rows nk*128+p
    wt = np.ascontiguousarray(
        wt32.reshape(NK, 128, 128).transpose(1, 0, 2).reshape(128, -1)
    ).astype(ml_dtypes.float8_e3m4)
    wt0 = np.ascontiguousarray(wt[:, :8 * 128])
    wt1 = np.ascontiguousarray(wt[:, 8 * 128:])
    bb = np.stack([np.concatenate([noise_b, expert_b]),
                   np.concatenate([0 * noise_b, -expert_b])],
                  axis=1).astype(np.float32)
    in_maps = []
    for c in range(NCORES):
        xs = np.ascontiguousarray(x[c * BC:(c + 1) * BC, :].T).astype(
            ml_dtypes.float8_e3m4)
        # per tile: [D, bt] -> [128, NK, bt], concatenated flat
        blocks = []
        for t, bt in enumerate(TILES):
            blk = xs[:, OFFS[t]:OFFS[t] + bt].reshape(NK, 128, bt)
            blocks.append(blk.transpose(1, 0, 2).reshape(-1))
        xr = np.ascontiguousarray(np.concatenate(blocks))
        in_maps.append({"xt": xr, "wt0": wt0, "wt1": wt1, "bb": bb})
    return in_maps


def kernel(x, noise, router_w, router_b, noise_w, noise_b, expert_w, expert_b,
           _trace=False):
    from concourse.bass_utils import run_bass_kernel_spmd

    x = np.asarray(x, dtype=np.float32)
    nc = get_program()
    in_maps = make_in_maps(x, np.asarray(noise_w), np.asarray(noise_b),
                           np.asarray(expert_w), np.asarray(expert_b))
    res = run_bass_kernel_spmd(nc, in_maps, core_ids=list(range(NCORES)),
                               trace=_trace)
    out = np.concatenate([r["out"] for r in res.results], axis=0)
    if _trace:
        kernel.last_results = res
    return out


# revision 66
# speedup vs baseline: 1.4499x; 1.0220x over previous
"""MoE logistic regression kernel for 8 Trainium2 NeuronCores.

Math (after dead-code elimination of the reference's unused router path):
    noise_logits = x @ noise_w.T + noise_b            # [B, E]
    top8 = top_k(noise_logits, 8)
    gates = softmax over the top-8 entries (others 0)
    expert = sigmoid(x @ expert_w.T + expert_b)       # [B, E]
    out[b] = sum_e gates[b,e] * expert[b,e]           # [B, 1]

Sharding: batch split 8 ways (2048 rows/core); weights replicated.

Implementation notes:
- x streams in fp8 (e3m4), weights in fp8 (e3m4, x512 host prescale so the
  U(-1/64,1/64) weights land in e3m4's normal range; the 1/512 descale
  folds into the ACT activation scale for free). Halves both the DMA
  bytes and keeps the PE at 1 cycle/row. End-to-end l2 rel err ~1.3e-2
  vs the 2e-2 gate (top-8 selection flips on ~2k/16384 rows dominate).
- Batch-tile-major stream: each tile's full contraction arrives while
  the previous tile's epilogue runs on ACT/DVE. PE is the critical
  resource (65536 matmul cycles ~27.3us at 2.4GHz); the shared DMA pipe
  carries ~27us (x 25.3us + fp8 weights 1.6us).
- PE p-state warm-up: dummy matmuls burn the 0.65/1.2GHz ramp before the
  first real matmul so the real stream runs at 2.4GHz throughout.
- Fused epilogue: exp(+/-(z*s+b)) comes straight off PSUM on ACT (one op
  per half), transposes to batch-major on PE, and the whole gating chain
  runs on DVE: top-8 via Max8, zsum/s4 via two independent accumulating
  scalar_tensor_tensor selects (es = e*sigmoid precomputed), final
  s4/zsum. All ACT ops stay in the one exp_and_others function set.
- Per-tile outputs DMA straight from the [128, j] result; early tiles
  batch through final_sb on the idle gpsimd SWDGE path.
"""

import sys

import numpy as np

if "/opt/trn_rl_repo" not in sys.path:
    sys.path.insert(0, "/opt/trn_rl_repo")

B, D, E, TOPK, NCORES = 16384, 4096, 64, 8, 8
BC = B // NCORES      # batch rows per core
NK = D // 128         # contraction chunks
WSCALE = 512.0        # host weight prescale (descaled in the epilogue)
# Tiles big-to-small: the per-tile epilogue chains must hide inside the
# following tiles' matmul windows; only the last (small) tile's chain is
# exposed as tail latency.
TILES = [512, 512, 512, 256, 128, 128]          # batch tile widths
OFFS = [sum(TILES[:i]) for i in range(len(TILES))]
assert sum(TILES) == BC
# DMA grouping in k-chunks per tile (each DMA costs ~650ns of queue
# issue+generation, so groups stay >= 4 chunks); tile 0 leads with
# 4-chunk groups so the first real matmul starts early, the final tile
# tapers so almost no matmul work remains after the last byte lands
GROUPS = [[4, 4, 4, 4, 8, 8]] + [[8, 8, 8, 8]] * 4 + [[16, 8, 4, 2, 2]]

_cached = {}


def _build_program():
    import concourse.bass as bass
    import concourse.tile as tile
    from concourse import bacc, mybir
    from concourse.masks import make_identity

    f32 = mybir.dt.float32
    f16 = mybir.dt.float16
    f8e3 = mybir.dt.float8e3
    act = mybir.ActivationFunctionType
    alu = mybir.AluOpType

    nc = bacc.Bacc("TRN2", target_bir_lowering=False, debug=False)
    # x fp8 (e3m4), per-tile partition-major blocks concatenated flat:
    # tile t occupies [128, NK, bt] at element offset 128*NK*OFFS[t], so
    # every group DMA is one contiguous gsz*bt-byte run per partition.
    xt = nc.dram_tensor("xt", [NK * 128 * BC], f8e3, kind="ExternalInput").ap()
    wt0 = nc.dram_tensor("wt0", [128, 8 * 128], f8e3, kind="ExternalInput").ap()
    wt1 = nc.dram_tensor("wt1", [128, (NK - 8) * 128], f8e3,
                         kind="ExternalInput").ap()
    bb = nc.dram_tensor("bb", [128, 2], f32, kind="ExternalInput").ap()
    out = nc.dram_tensor("out", [BC, 1], f32, kind="ExternalOutput").ap()

    with tile.TileContext(nc) as tc:
        with (
            tc.tile_pool(name="consts", bufs=1) as consts,
            tc.tile_pool(name="xpool", bufs=6) as xpool,
            tc.tile_pool(name="eppool", bufs=4) as eppool,
            tc.tile_pool(name="small", bufs=4) as small,
            tc.tile_pool(name="tvp", bufs=8) as tvp,
            tc.tile_pool(name="psacc", bufs=1, space=bass.MemorySpace.PSUM) as psacc,
            tc.tile_pool(name="pstr", bufs=2, space=bass.MemorySpace.PSUM) as pstr,
        ):
            # ---- constants ----
            accs = [psacc.tile([128, 512], f32, tag=f"acc{t}", name=f"acc{t}")
                    for t in range(len(TILES))]
            # PE p-state warm-up: matmul speed ramps 0.65->1.2->2.4 GHz only
            # after ~3us of continuous PE execution. Burn the ramp on dummy
            # 64-col matmuls (junk scratch via a ~0.9us DVE memset, no DMA
            # deps) so every real matmul runs at full clock. The last acc's
            # first real matmul start=True resets the bank.
            junk = consts.tile([128, 64], f32)
            nc.vector.memset(junk, 0.0)
            for wi in range(14):
                nc.tensor.matmul(accs[-1][0:64, 0:64], lhsT=junk, rhs=junk,
                                 start=True, stop=True)
            # w0 rides the Pool SWDGE queue: its transfer overlaps the SP
            # x-stream pipe start, landing in time for the first real
            # matmul without displacing the first x group.
            w0_sb = consts.tile([128, 8, 128], f8e3)
            nc.gpsimd.dma_start(out=w0_sb,
                                in_=wt0.rearrange("p (g m) -> p g m", g=8))
            bb_sb = consts.tile([128, 2], f32)
            nc.gpsimd.dma_start(out=bb_sb, in_=bb)
            # w1 split in three on ACT so each piece slots between x groups
            # on the shared transfer pipe, landing just before the matmuls
            # that need it.
            w1_sb = consts.tile([128, NK - 8, 128], f8e3)
            w1r = wt1.rearrange("p (g m) -> p g m", g=NK - 8)
            nc.scalar.dma_start(out=w1_sb[:, 0:8, :], in_=w1r[:, 0:8, :])
            nc.scalar.dma_start(out=w1_sb[:, 8:16, :], in_=w1r[:, 8:16, :])
            nc.scalar.dma_start(out=w1_sb[:, 16:24, :], in_=w1r[:, 16:24, :])
            # fp16 identity: transposes of fp16 data cost 1 PE cycle/row
            # (fp32 costs 2). The fp32 one serves tiles 4/5, whose staging
            # lives in retired fp32 acc banks.
            ident = consts.tile([128, 128], f16)
            make_identity(nc, ident)
            ident32 = consts.tile([64, 64], f32)
            make_identity(nc, ident32)
            # warm the ACT exp_and_others table during the DMA phase; every
            # later ACT op (Identity/Copy/Exp) stays in this one set.
            warm = consts.tile([1, 1], f32)
            nc.vector.memset(warm, 0.0)
            nc.scalar.add(warm, warm, bb_sb[0:1, 0:1])
            nc.scalar.activation(warm, warm, func=act.Exp)

            for t, bt in enumerate(TILES):
                njs = bt // 128
                off = OFFS[t]
                acc = accs[t][:, 0:bt]
                # ---- stream tile t's contraction, accumulate logits.T ----
                # acc[0:64,:] = WSCALE*noise logits.T, acc[64:128,:] =
                # WSCALE*expert logits.T (both pre-bias)
                base = 128 * NK * off
                xtile = xt[base:base + 128 * NK * bt].rearrange(
                    "(p k b) -> p k b", p=128, k=NK)
                k0 = 0
                for gsz in GROUPS[t]:
                    xk = xpool.tile([128, gsz, bt], f8e3, tag=f"xk{bt}_{gsz}")
                    nc.sync.dma_start(out=xk, in_=xtile[:, k0:k0 + gsz, :])
                    for g in range(gsz):
                        k = k0 + g
                        w = w0_sb[:, k, :] if k < 8 else w1_sb[:, k - 8, :]
                        nc.tensor.matmul(acc, lhsT=w, rhs=xk[:, g, :],
                                         start=(k == 0), stop=(k == NK - 1))
                    k0 += gsz

                # ---- epilogue for tile t (overlaps tile t+1's stream) ----
                # eA = exp(noise_logit + nb), eB = exp(-(expert_logit + eb)),
                # both straight off PSUM with the 1/WSCALE descale folded
                # in. eB FIRST: the longest chain runs through
                # eB->transpose->den->sig->es, so it must clear ACT first.
                ed = f16 if t < 4 else f32
                sfx = "16" if t < 4 else "32"
                eB = eppool.tile([64, bt], ed, tag=f"eB{bt}{sfx}")
                nc.scalar.activation(eB, accs[t][64:128, 0:bt], func=act.Exp,
                                     scale=-1.0 / WSCALE,
                                     bias=bb_sb[64:128, 1:2])
                eA = eppool.tile([64, bt], ed, tag=f"eA{bt}{sfx}")
                nc.scalar.activation(eA, accs[t][0:64, 0:bt], func=act.Exp,
                                     scale=1.0 / WSCALE,
                                     bias=bb_sb[0:64, 0:1])
                # transpose to batch-major: [128 batch, j | 4+j, 64];
                # expert half first (critical chain). Tiles 0-3 stage fp16
                # (half the PE transpose cost); tiles 4/5 stage fp32 in the
                # long-retired acc0/acc1 banks so they never wait on the
                # 2-deep pstr rotation.
                if t < 4:
                    ps = pstr.tile([128, 8, 64], f16, tag="ps", name=f"ps{t}")
                    psA = [ps[:, j, :] for j in range(njs)]
                    psB = [ps[:, 4 + j, :] for j in range(njs)]
                    psBall = ps[:, 4:4 + njs, :]
                    idt = ident
                else:
                    psA = [accs[t - 4][:, j * 64:(j + 1) * 64]
                           for j in range(njs)]
                    psB = [accs[t - 4][:, 256 + j * 64:256 + (j + 1) * 64]
                           for j in range(njs)]
                    psBall = accs[t - 4][:, 256:256 + njs * 64]
                    idt = ident32
                for j in range(njs):
                    nc.tensor.transpose(psB[j],
                                        eB[:, j * 128:(j + 1) * 128],
                                        idt[0:64, 0:64])
                for j in range(njs):
                    nc.tensor.transpose(psA[j],
                                        eA[:, j * 128:(j + 1) * 128],
                                        idt[0:64, 0:64])
                # den = 1 + eB; sigmoid = 1/den -- emitted before tv/zred so
                # the sig chain (which es and the final select depend on)
                # clears DVE first
                den = small.tile([128, 4, 64], ed, tag=f"den{sfx}")
                nc.vector.tensor_scalar_add(den[:, 0:njs, :], psBall, 1.0)
                sig = small.tile([128, 4, 64], ed, tag=f"sig{sfx}")
                with nc.allow_low_precision(reason="sigmoid in (0,1): fp16 "
                                            "rel err ~5e-4 vs 1.3e-2 l2"):
                    nc.vector.reciprocal(sig[:, 0:njs, :], den[:, 0:njs, :])
                # top-8 on exp(v) (monotone => same selection as on v);
                # zsum = sum of the top-8 values in ONE reduce over tv
                tv = tvp.tile([128, 32], ed, tag=f"tv{sfx}", name=f"tv{t}")
                for j in range(njs):
                    nc.vector.max(tv[:, j * 8:(j + 1) * 8], psA[j])
                zsum = small.tile([128, 4], f32, tag="zsum")
                nc.vector.tensor_reduce(
                    zsum[:, 0:njs],
                    tv.rearrange("p (j k) -> p j k", k=8)[:, 0:njs, :],
                    axis=mybir.AxisListType.X, op=alu.add)
                # es = e * sigmoid. All ps reads stay on DVE so the PSUM
                # staging buffer frees as soon as the DVE chain drains.
                es = small.tile([128, 4, 64], ed, tag=f"es{sfx}")
                for j in range(njs):
                    nc.vector.tensor_mul(es[:, j, :], psA[j], sig[:, j, :])
                # s4 = sum of top-8 e*sigmoid (accumulating select)
                s4 = small.tile([128, 4], f32, tag="s4")
                scr = small.tile([128, 4, 64], ed, tag=f"scr{sfx}")
                for j in range(njs):
                    nc.vector.scalar_tensor_tensor(
                        out=scr[:, j, :], in0=psA[j],
                        scalar=tv[:, j * 8 + 7:j * 8 + 8], in1=es[:, j, :],
                        op0=alu.is_ge, op1=alu.mult,
                        accum_out=s4[:, j:j + 1])
                rz = small.tile([128, 4], f32, tag="rz")
                nc.vector.reciprocal(rz[:, 0:njs], zsum[:, 0:njs])
                fin = small.tile([128, 4], f32, tag="fin")
                nc.vector.tensor_mul(fin[:, 0:njs], s4[:, 0:njs],
                                     rz[:, 0:njs])
                # outputs ride the gpsimd SWDGE queue mid-stream (Pool is
                # idle once the weight gens finish; the ACT queue is
                # in-order, so an out issue there would block the next
                # tile's eA). The last tile uses SP, idle once the x stream
                # is issued.
                out_t = out[off:off + bt, :].rearrange(
                    "(j p) o -> p (j o)", j=njs, p=128)
                eng = nc.sync if t == len(TILES) - 1 else nc.gpsimd
                eng.dma_start(out=out_t, in_=fin[:, 0:njs])

    nc.compile()
    return nc


def get_program():
    if "prog" not in _cached:
        _cached["prog"] = _build_program()
    return _cached["prog"]


def make_in_maps(x, noise_w, noise_b, expert_w, expert_b):
    """Host-side sharding: per-core transposed fp8(e3m4) x + weights."""
    import ml_dtypes
    w_comb = np.concatenate([noise_w, expert_w], axis=0).astype(np.float32)
    wt32 = np.ascontiguousarray(w_comb.T) * np.float32(WSCALE)   # [D, 128]
    # partition p holds [nk, 128] for contraction rows nk*128+p
    wt = np.ascontiguousarray(
        wt32.reshape(NK, 128, 128).transpose(1, 0, 2).reshape(128, -1)
    ).astype(ml_dtypes.float8_e3m4)
    wt0 = np.ascontiguousarray(wt[:, :8 * 128])
    wt1 = np.ascontiguousarray(wt[:, 8 * 128:])
    bb = np.stack([np.concatenate([noise_b, expert_b]),
                   np.concatenate([0 * noise_b, -expert_b])],
                  axis=1).astype(np.float32)
    in_maps = []
    for c in range(NCORES):
        xs = np.ascontiguousarray(x[c * BC:(c + 1) * BC, :].T).astype(
            ml_dtypes.float8_e3m4)
        # per tile: [D, bt] -> [128, NK, bt], concatenated flat
        blocks = []
        for t, bt in enumerate(TILES):
            blk = xs[:, OFFS[t]:OFFS[t] + bt].reshape(NK, 128, bt)
            blocks.append(blk.transpose(1, 0, 2).reshape(-1))
        xr = np.ascontiguousarray(np.concatenate(blocks))
        in_maps.append({"xt": xr, "wt0": wt0, "wt1": wt1, "bb": bb})
    return in_maps


def kernel(x, noise, router_w, router_b, noise_w, noise_b, expert_w, expert_b,
           _trace=False):
    from concourse.bass_utils import run_bass_kernel_spmd

    x = np.asarray(x, dtype=np.float32)
    nc = get_program()
    in_maps = make_in_maps(x, np.asarray(noise_w), np.asarray(noise_b),
                           np.asarray(expert_w), np.asarray(expert_b))
    res = run_bass_kernel_spmd(nc, in_maps, core_ids=list(range(NCORES)),
                               trace=_trace)
    out = np.concatenate([r["out"] for r in res.results], axis=0)
    if _trace:
        kernel.last_results = res
    return out


# revision 75
# speedup vs baseline: 1.4838x; 1.0234x over previous
"""MoE logistic regression kernel for 8 Trainium2 NeuronCores.

Math (after dead-code elimination of the reference's unused router path):
    noise_logits = x @ noise_w.T + noise_b            # [B, E]
    top8 = top_k(noise_logits, 8)
    gates = softmax over the top-8 entries (others 0)
    expert = sigmoid(x @ expert_w.T + expert_b)       # [B, E]
    out[b] = sum_e gates[b,e] * expert[b,e]           # [B, 1]

Sharding: batch split 8 ways (2048 rows/core); weights replicated.

Implementation notes:
- x streams in fp8 (e3m4), weights in fp8 (e3m4, x512 host prescale so the
  U(-1/64,1/64) weights land in e3m4's normal range; the 1/512 descale
  folds into the ACT activation scale for free). Halves both the DMA
  bytes and keeps the PE at 1 cycle/row. End-to-end l2 rel err ~1.3e-2
  vs the 2e-2 gate (top-8 selection flips on ~2k/16384 rows dominate).
- Batch-tile-major stream: each tile's full contraction arrives while
  the previous tile's epilogue runs on ACT/DVE. PE is the critical
  resource (65536 matmul cycles ~27.3us at 2.4GHz); the shared DMA pipe
  carries ~27us (x 25.3us + fp8 weights 1.6us).
- PE p-state warm-up: dummy matmuls burn the 0.65/1.2GHz ramp before the
  first real matmul so the real stream runs at 2.4GHz throughout.
- Fused epilogue: exp(+/-(z*s+b)) comes straight off PSUM on ACT (one op
  per half), transposes to batch-major on PE, and the whole gating chain
  runs on DVE: top-8 via Max8, zsum/s4 via two independent accumulating
  scalar_tensor_tensor selects (es = e*sigmoid precomputed), final
  s4/zsum. All ACT ops stay in the one exp_and_others function set.
- Per-tile outputs DMA straight from the [128, j] result; early tiles
  batch through final_sb on the idle gpsimd SWDGE path.
"""

import sys

import numpy as np

if "/opt/trn_rl_repo" not in sys.path:
    sys.path.insert(0, "/opt/trn_rl_repo")

B, D, E, TOPK, NCORES = 16384, 4096, 64, 8, 8
BC = B // NCORES      # batch rows per core
NK = D // 128         # contraction chunks
WSCALE = 512.0        # host weight prescale (descaled in the epilogue)
# Tiles big-to-small: the per-tile epilogue chains must hide inside the
# following tiles' matmul windows; only the last (small) tile's chain is
# exposed as tail latency.
TILES = [512, 512, 512, 256, 128, 128]          # batch tile widths
OFFS = [sum(TILES[:i]) for i in range(len(TILES))]
assert sum(TILES) == BC
# DMA grouping in k-chunks per tile (each DMA costs ~650ns of queue
# issue+generation, so groups stay >= 4 chunks); tile 0 leads with
# 4-chunk groups so the first real matmul starts early, the final tile
# tapers so almost no matmul work remains after the last byte lands
GROUPS = [[4, 4, 4, 4, 8, 8]] + [[8, 8, 8, 8]] * 4 + [[16, 8, 4, 2, 2]]

_cached = {}


def _build_program():
    import concourse.bass as bass
    import concourse.tile as tile
    from concourse import bacc, mybir
    from concourse.masks import make_identity

    f32 = mybir.dt.float32
    f16 = mybir.dt.float16
    f8e3 = mybir.dt.float8e3
    act = mybir.ActivationFunctionType
    alu = mybir.AluOpType

    nc = bacc.Bacc("TRN2", target_bir_lowering=False, debug=False)
    # x fp8 (e3m4), per-tile partition-major blocks concatenated flat:
    # tile t occupies [128, NK, bt] at element offset 128*NK*OFFS[t], so
    # every group DMA is one contiguous gsz*bt-byte run per partition.
    xt = nc.dram_tensor("xt", [NK * 128 * BC], f8e3, kind="ExternalInput").ap()
    wt0 = nc.dram_tensor("wt0", [128, 8 * 128], f8e3, kind="ExternalInput").ap()
    wt1 = nc.dram_tensor("wt1", [128, (NK - 8) * 128], f8e3,
                         kind="ExternalInput").ap()
    bb = nc.dram_tensor("bb", [128, 1], f32, kind="ExternalInput").ap()
    out = nc.dram_tensor("out", [BC, 1], f32, kind="ExternalOutput").ap()

    with tile.TileContext(nc) as tc:
        with (
            tc.tile_pool(name="consts", bufs=1) as consts,
            tc.tile_pool(name="xpool", bufs=6) as xpool,
            tc.tile_pool(name="eppool", bufs=4) as eppool,
            tc.tile_pool(name="small", bufs=4) as small,
            tc.tile_pool(name="tvp", bufs=8) as tvp,
            tc.tile_pool(name="psacc", bufs=1, space=bass.MemorySpace.PSUM) as psacc,
            tc.tile_pool(name="pstr", bufs=2, space=bass.MemorySpace.PSUM) as pstr,
        ):
            # ---- constants ----
            accs = [psacc.tile([128, 512], f32, tag=f"acc{t}", name=f"acc{t}")
                    for t in range(len(TILES))]
            # PE p-state warm-up: matmul speed ramps 0.65->1.2->2.4 GHz only
            # after ~3us of continuous PE execution. Burn the ramp on dummy
            # 64-col matmuls (junk scratch via a Pool memset at ~60ns, no
            # DMA deps) so every real matmul runs at full clock. The last
            # acc's first real matmul start=True resets the bank.
            junk = consts.tile([128, 64], f32)
            nc.gpsimd.memset(junk, 0.0)
            for wi in range(16):
                nc.tensor.matmul(accs[-1][0:64, 0:64], lhsT=junk, rhs=junk,
                                 start=True, stop=True)
            # w0 rides the Pool SWDGE queue: its transfer overlaps the SP
            # x-stream pipe start, landing in time for the first real
            # matmul without displacing the first x group.
            w0_sb = consts.tile([128, 8, 128], f8e3)
            nc.gpsimd.dma_start(out=w0_sb,
                                in_=wt0.rearrange("p (g m) -> p g m", g=8))
            bb_sb = consts.tile([128, 1], f32)
            nc.gpsimd.dma_start(out=bb_sb, in_=bb)
            # w1 split in three on ACT so each piece slots between x groups
            # on the shared transfer pipe, landing just before the matmuls
            # that need it.
            w1_sb = consts.tile([128, NK - 8, 128], f8e3)
            w1r = wt1.rearrange("p (g m) -> p g m", g=NK - 8)
            nc.scalar.dma_start(out=w1_sb[:, 0:8, :], in_=w1r[:, 0:8, :])
            nc.scalar.dma_start(out=w1_sb[:, 8:16, :], in_=w1r[:, 8:16, :])
            nc.scalar.dma_start(out=w1_sb[:, 16:24, :], in_=w1r[:, 16:24, :])
            # fp16 identity: transposes of fp16 data cost 1 PE cycle/row
            # (fp32 costs 2). The fp32 one serves tiles 4/5, whose staging
            # lives in retired fp32 acc banks.
            ident = consts.tile([128, 128], f16)
            make_identity(nc, ident)
            ident32 = consts.tile([128, 128], f32)
            make_identity(nc, ident32)
            # warm the ACT exp_and_others table during the DMA phase; every
            # later ACT op (Identity/Copy/Exp) stays in this one set.
            warm = consts.tile([1, 1], f32)
            nc.vector.memset(warm, 0.0)
            nc.scalar.add(warm, warm, bb_sb[0:1, 0:1])
            nc.scalar.activation(warm, warm, func=act.Exp)

            for t, bt in enumerate(TILES):
                njs = bt // 128
                off = OFFS[t]
                acc = accs[t][:, 0:bt]
                # ---- stream tile t's contraction, accumulate logits.T ----
                # acc[0:64,:] = WSCALE*noise logits.T, acc[64:128,:] =
                # WSCALE*expert logits.T (both pre-bias)
                base = 128 * NK * off
                xtile = xt[base:base + 128 * NK * bt].rearrange(
                    "(p k b) -> p k b", p=128, k=NK)
                k0 = 0
                for gsz in GROUPS[t]:
                    xk = xpool.tile([128, gsz, bt], f8e3, tag=f"xk{bt}_{gsz}")
                    nc.sync.dma_start(out=xk, in_=xtile[:, k0:k0 + gsz, :])
                    for g in range(gsz):
                        k = k0 + g
                        w = w0_sb[:, k, :] if k < 8 else w1_sb[:, k - 8, :]
                        nc.tensor.matmul(acc, lhsT=w, rhs=xk[:, g, :],
                                         start=(k == 0), stop=(k == NK - 1))
                    k0 += gsz

                # ---- epilogue for tile t (overlaps tile t+1's stream) ----
                # ONE exp for both halves straight off PSUM: the expert
                # weights/bias are host-negated, so e[0:64] = exp(nz+nb) and
                # e[64:128] = exp(-(ez+eb)) share the same +1/WSCALE scale.
                ed = f16 if t < 4 else f32
                sfx = "16" if t < 4 else "32"
                ecomb = eppool.tile([128, bt], ed, tag=f"ec{bt}{sfx}")
                nc.scalar.activation(ecomb, accs[t][:, 0:bt], func=act.Exp,
                                     scale=1.0 / WSCALE, bias=bb_sb)
                # transpose to batch-major [128 batch, j, 0:64|64:128] in
                # one [128,128] transpose per j-block. Tiles 0-3 stage fp16
                # (half the PE transpose cost); tiles 4/5 stage fp32 in the
                # long-retired acc0/acc1 banks so they never wait on the
                # 2-deep pstr rotation.
                if t < 4:
                    ps = pstr.tile([128, 4, 128], f16, tag="ps",
                                   name=f"ps{t}")
                    psC = [ps[:, j, :] for j in range(njs)]
                    psA = [ps[:, j, 0:64] for j in range(njs)]
                    psAall = ps[:, 0:njs, 0:64]
                    psBall = ps[:, 0:njs, 64:128]
                    idt = ident
                else:
                    psC = [accs[t - 4][:, j * 128:(j + 1) * 128]
                           for j in range(njs)]
                    psA = [accs[t - 4][:, j * 128:j * 128 + 64]
                           for j in range(njs)]
                    psAall = psA[0]
                    psBall = accs[t - 4][:, 64:128]
                    idt = ident32
                for j in range(njs):
                    nc.tensor.transpose(psC[j],
                                        ecomb[:, j * 128:(j + 1) * 128],
                                        idt)
                # den = 1 + eB; sigmoid = 1/den -- emitted before tv/zred so
                # the sig chain (which es and the final select depend on)
                # clears DVE first
                # den/es for the last two tiles run on the idle Pool engine
                # (their staging is in retired acc banks, so Pool reading ps
                # cannot stall later transposes); this keeps tile 5's DVE
                # chain from queueing behind tile 4's.
                ve = nc.vector if t < 4 else nc.gpsimd
                den = small.tile([128, 4, 64], ed, tag=f"den{sfx}")
                ve.tensor_scalar_add(den[:, 0:njs, :], psBall, 1.0)
                sig = small.tile([128, 4, 64], ed, tag=f"sig{sfx}")
                with nc.allow_low_precision(reason="sigmoid in (0,1): fp16 "
                                            "rel err ~5e-4 vs 1.3e-2 l2"):
                    nc.vector.reciprocal(sig[:, 0:njs, :], den[:, 0:njs, :])
                # top-8 on exp(v) (monotone => same selection as on v);
                # zsum = sum of the top-8 values in ONE reduce over tv
                tv = tvp.tile([128, 32], ed, tag=f"tv{sfx}", name=f"tv{t}")
                for j in range(njs):
                    nc.vector.max(tv[:, j * 8:(j + 1) * 8], psA[j])
                zsum = small.tile([128, 4], f32, tag="zsum")
                nc.vector.tensor_reduce(
                    zsum[:, 0:njs],
                    tv.rearrange("p (j k) -> p j k", k=8)[:, 0:njs, :],
                    axis=mybir.AxisListType.X, op=alu.add)
                # es = e * sigmoid in ONE strided op. All ps reads stay on
                # DVE so the PSUM staging buffer frees as soon as the DVE
                # chain drains.
                es = small.tile([128, 4, 64], ed, tag=f"es{sfx}")
                ve.tensor_mul(es[:, 0:njs, :], psAall, sig[:, 0:njs, :])
                # s4 = sum of top-8 e*sigmoid (accumulating select)
                s4 = small.tile([128, 4], f32, tag="s4")
                scr = small.tile([128, 4, 64], ed, tag=f"scr{sfx}")
                for j in range(njs):
                    nc.vector.scalar_tensor_tensor(
                        out=scr[:, j, :], in0=psA[j],
                        scalar=tv[:, j * 8 + 7:j * 8 + 8], in1=es[:, j, :],
                        op0=alu.is_ge, op1=alu.mult,
                        accum_out=s4[:, j:j + 1])
                rz = small.tile([128, 4], f32, tag="rz")
                nc.vector.reciprocal(rz[:, 0:njs], zsum[:, 0:njs])
                fin = small.tile([128, 4], f32, tag="fin")
                nc.vector.tensor_mul(fin[:, 0:njs], s4[:, 0:njs],
                                     rz[:, 0:njs])
                # outputs ride the ACT HWDGE queue mid-stream (ACT only has
                # one ecomb per tile now, and HWDGE gen runs off-engine;
                # Pool must stay free for the last tiles' den/es). The last
                # tile uses SP, idle once the x stream is issued.
                out_t = out[off:off + bt, :].rearrange(
                    "(j p) o -> p (j o)", j=njs, p=128)
                eng = nc.sync if t == len(TILES) - 1 else nc.scalar
                eng.dma_start(out=out_t, in_=fin[:, 0:njs])

    nc.compile()
    return nc


def get_program():
    if "prog" not in _cached:
        _cached["prog"] = _build_program()
    return _cached["prog"]


def make_in_maps(x, noise_w, noise_b, expert_w, expert_b):
    """Host-side sharding: per-core transposed fp8(e3m4) x + weights.

    The expert half is NEGATED (weights and bias) so the kernel computes
    exp(+scale*acc + bias) for all 128 logit rows in one ACT op:
    rows 64:128 then hold exp(-(expert_logit + expert_b)) directly.
    """
    import ml_dtypes
    w_comb = np.concatenate([noise_w, -expert_w], axis=0).astype(np.float32)
    wt32 = np.ascontiguousarray(w_comb.T) * np.float32(WSCALE)   # [D, 128]
    # partition p holds [nk, 128] for contraction rows nk*128+p
    wt = np.ascontiguousarray(
        wt32.reshape(NK, 128, 128).transpose(1, 0, 2).reshape(128, -1)
    ).astype(ml_dtypes.float8_e3m4)
    wt0 = np.ascontiguousarray(wt[:, :8 * 128])
    wt1 = np.ascontiguousarray(wt[:, 8 * 128:])
    bb = np.concatenate([noise_b, -expert_b]).reshape(128, 1).astype(
        np.float32)
    in_maps = []
    for c in range(NCORES):
        xs = np.ascontiguousarray(x[c * BC:(c + 1) * BC, :].T).astype(
            ml_dtypes.float8_e3m4)
        # per tile: [D, bt] -> [128, NK, bt], concatenated flat
        blocks = []
        for t, bt in enumerate(TILES):
            blk = xs[:, OFFS[t]:OFFS[t] + bt].reshape(NK, 128, bt)
            blocks.append(blk.transpose(1, 0, 2).reshape(-1))
        xr = np.ascontiguousarray(np.concatenate(blocks))
        in_maps.append({"xt": xr, "wt0": wt0, "wt1": wt1, "bb": bb})
    return in_maps


def kernel(x, noise, router_w, router_b, noise_w, noise_b, expert_w, expert_b,
           _trace=False):
    from concourse.bass_utils import run_bass_kernel_spmd

    x = np.asarray(x, dtype=np.float32)
    nc = get_program()
    in_maps = make_in_maps(x, np.asarray(noise_w), np.asarray(noise_b),
                           np.asarray(expert_w), np.asarray(expert_b))
    res = run_bass_kernel_spmd(nc, in_maps, core_ids=list(range(NCORES)),
                               trace=_trace)
    out = np.concatenate([r["out"] for r in res.results], axis=0)
    if _trace:
        kernel.last_results = res
    return out


# revision 82
# speedup vs baseline: 1.4914x; 1.0051x over previous
"""MoE logistic regression kernel for 8 Trainium2 NeuronCores.

Math (after dead-code elimination of the reference's unused router path):
    noise_logits = x @ noise_w.T + noise_b            # [B, E]
    top8 = top_k(noise_logits, 8)
    gates = softmax over the top-8 entries (others 0)
    expert = sigmoid(x @ expert_w.T + expert_b)       # [B, E]
    out[b] = sum_e gates[b,e] * expert[b,e]           # [B, 1]

Sharding: batch split 8 ways (2048 rows/core); weights replicated.

Implementation notes:
- x streams in fp8 (e3m4), weights in fp8 (e3m4, x512 host prescale so
  the U(-1/64,1/64) weights land in e3m4's normal range; the 1/512
  descale folds into the ACT activation scale for free). Halves the DMA
  bytes and keeps the PE at 1 cycle/row. End-to-end l2 rel err ~1.5e-2
  vs the 2e-2 gate (top-8 selection flips on ~2.2k/16384 rows dominate).
- Batch-tile-major stream: each tile's full contraction arrives while
  the previous tile's epilogue runs. PE is the critical resource (65536
  matmul cycles ~27.3us at 2.4GHz); the shared serial DMA transfer pipe
  carries ~27us (x 25.3us + fp8 weights 1.6us) and starts ~2us in, so
  PE trails it by design. Every DMA costs ~650ns of queue issue + HWDGE
  generation, hence >=8-chunk groups.
- PE p-state warm-up: dummy matmuls (junk SBUF, Pool-memset at ~60ns)
  burn the 0.65/1.2GHz ramp so real matmuls run at 2.4GHz from the
  start.
- The expert half of the weights/bias is HOST-NEGATED, so ONE ACT exp
  per tile produces e[0:64]=exp(noise_logit+nb) and
  e[64:128]=exp(-(expert_logit+eb)) straight off PSUM, and ONE
  [128,128] PE transpose per 128-col block lands both halves
  batch-major. fp16 staging for tiles 0-3 halves the PE transpose cost
  (fp16 collisions at the top-8 boundary add ~2e-3 l2); tiles 4/5 stage
  fp32 in long-retired acc banks to dodge the 2-deep pstr rotation.
- DVE gating chain per tile: den=1+eB, sig=recip, Max8 per 128-block,
  zsum = one tensor_reduce over the Max8 outputs, es = e*sig in one
  strided mul, s4 via accumulating scalar_tensor_tensor selects, final
  s4 * recip(zsum). All ACT ops stay in the exp_and_others table set so
  no mid-stream LoadActFuncSet ever fires; ps stays DVE-only-read so
  its PSUM buffer frees as soon as the chain drains.
- Per-tile outputs DMA from the [128, njs] result on the gpsimd SWDGE
  queue (off the x stream); the last tile's rides SP, idle by then.
"""

import sys

import numpy as np

if "/opt/trn_rl_repo" not in sys.path:
    sys.path.insert(0, "/opt/trn_rl_repo")

B, D, E, TOPK, NCORES = 16384, 4096, 64, 8, 8
BC = B // NCORES      # batch rows per core
NK = D // 128         # contraction chunks
WSCALE = 512.0        # host weight prescale (descaled in the epilogue)
# Tiles big-to-small: the per-tile epilogue chains must hide inside the
# following tiles' matmul windows; only the last (small) tile's chain is
# exposed as tail latency.
TILES = [512, 512, 512, 256, 128, 128]          # batch tile widths
OFFS = [sum(TILES[:i]) for i in range(len(TILES))]
assert sum(TILES) == BC
# DMA grouping in k-chunks per tile (each DMA costs ~650ns of queue
# issue+generation, so groups stay >= 4 chunks); tile 0 leads with
# 4-chunk groups so the first real matmul starts early, the final tile
# tapers so almost no matmul work remains after the last byte lands
GROUPS = [[8, 8, 8, 8]] * 5 + [[16, 8, 4, 2, 2]]

_cached = {}


def _build_program():
    import concourse.bass as bass
    import concourse.tile as tile
    from concourse import bacc, mybir
    from concourse.masks import make_identity

    f32 = mybir.dt.float32
    f16 = mybir.dt.float16
    f8e3 = mybir.dt.float8e3
    act = mybir.ActivationFunctionType
    alu = mybir.AluOpType

    nc = bacc.Bacc("TRN2", target_bir_lowering=False, debug=False)
    # x fp8 (e3m4), per-tile partition-major blocks concatenated flat:
    # tile t occupies [128, NK, bt] at element offset 128*NK*OFFS[t], so
    # every group DMA is one contiguous gsz*bt-byte run per partition.
    xt = nc.dram_tensor("xt", [NK * 128 * BC], f8e3, kind="ExternalInput").ap()
    wt0 = nc.dram_tensor("wt0", [128, 8 * 128], f8e3, kind="ExternalInput").ap()
    wt1 = nc.dram_tensor("wt1", [128, (NK - 8) * 128], f8e3,
                         kind="ExternalInput").ap()
    bb = nc.dram_tensor("bb", [128, 1], f32, kind="ExternalInput").ap()
    out = nc.dram_tensor("out", [BC, 1], f32, kind="ExternalOutput").ap()

    with tile.TileContext(nc) as tc:
        with (
            tc.tile_pool(name="consts", bufs=1) as consts,
            tc.tile_pool(name="xpool", bufs=6) as xpool,
            tc.tile_pool(name="eppool", bufs=4) as eppool,
            tc.tile_pool(name="small", bufs=4) as small,
            tc.tile_pool(name="tvp", bufs=8) as tvp,
            tc.tile_pool(name="psacc", bufs=1, space=bass.MemorySpace.PSUM) as psacc,
            tc.tile_pool(name="pstr", bufs=2, space=bass.MemorySpace.PSUM) as pstr,
        ):
            # ---- constants ----
            accs = [psacc.tile([128, 512], f32, tag=f"acc{t}", name=f"acc{t}")
                    for t in range(len(TILES))]
            # PE p-state warm-up: matmul speed ramps 0.65->1.2->2.4 GHz only
            # after ~3us of continuous PE execution. Burn the ramp on dummy
            # 64-col matmuls (junk scratch via a Pool memset at ~60ns, no
            # DMA deps) so every real matmul runs at full clock. The last
            # acc's first real matmul start=True resets the bank.
            junk = consts.tile([128, 64], f32)
            nc.gpsimd.memset(junk, 0.0)
            for wi in range(17):
                nc.tensor.matmul(accs[-1][0:64, 0:64], lhsT=junk, rhs=junk,
                                 start=True, stop=True)
            # w0 rides the Pool SWDGE queue: its transfer overlaps the SP
            # x-stream pipe start, landing in time for the first real
            # matmul without displacing the first x group.
            w0_sb = consts.tile([128, 8, 128], f8e3)
            nc.gpsimd.dma_start(out=w0_sb,
                                in_=wt0.rearrange("p (g m) -> p g m", g=8))
            bb_sb = consts.tile([128, 1], f32)
            nc.gpsimd.dma_start(out=bb_sb, in_=bb)
            # w1 split in three on ACT so each piece slots between x groups
            # on the shared transfer pipe, landing just before the matmuls
            # that need it.
            w1_sb = consts.tile([128, NK - 8, 128], f8e3)
            w1r = wt1.rearrange("p (g m) -> p g m", g=NK - 8)
            nc.scalar.dma_start(out=w1_sb[:, 0:8, :], in_=w1r[:, 0:8, :])
            nc.scalar.dma_start(out=w1_sb[:, 8:16, :], in_=w1r[:, 8:16, :])
            nc.scalar.dma_start(out=w1_sb[:, 16:24, :], in_=w1r[:, 16:24, :])
            # fp16 identity: transposes of fp16 data cost 1 PE cycle/row
            # (fp32 costs 2). The fp32 one serves tiles 4/5, whose staging
            # lives in retired fp32 acc banks.
            ident = consts.tile([128, 128], f16)
            make_identity(nc, ident)
            ident32 = consts.tile([128, 128], f32)
            make_identity(nc, ident32)
            # warm the ACT exp_and_others table during the DMA phase; every
            # later ACT op (Identity/Copy/Exp) stays in this one set.
            warm = consts.tile([1, 1], f32)
            nc.vector.memset(warm, 0.0)
            nc.scalar.add(warm, warm, bb_sb[0:1, 0:1])
            nc.scalar.activation(warm, warm, func=act.Exp)

            for t, bt in enumerate(TILES):
                njs = bt // 128
                off = OFFS[t]
                acc = accs[t][:, 0:bt]
                # ---- stream tile t's contraction, accumulate logits.T ----
                # acc[0:64,:] = WSCALE*noise logits.T, acc[64:128,:] =
                # WSCALE*expert logits.T (both pre-bias)
                base = 128 * NK * off
                xtile = xt[base:base + 128 * NK * bt].rearrange(
                    "(p k b) -> p k b", p=128, k=NK)
                k0 = 0
                for gsz in GROUPS[t]:
                    xk = xpool.tile([128, gsz, bt], f8e3, tag=f"xk{bt}_{gsz}")
                    nc.sync.dma_start(out=xk, in_=xtile[:, k0:k0 + gsz, :])
                    for g in range(gsz):
                        k = k0 + g
                        w = w0_sb[:, k, :] if k < 8 else w1_sb[:, k - 8, :]
                        nc.tensor.matmul(acc, lhsT=w, rhs=xk[:, g, :],
                                         start=(k == 0), stop=(k == NK - 1))
                    k0 += gsz

                # ---- epilogue for tile t (overlaps tile t+1's stream) ----
                # ONE exp for both halves straight off PSUM: the expert
                # weights/bias are host-negated, so e[0:64] = exp(nz+nb) and
                # e[64:128] = exp(-(ez+eb)) share the same +1/WSCALE scale.
                ed = f16 if t < 4 else f32
                sfx = "16" if t < 4 else "32"
                ecomb = eppool.tile([128, bt], ed, tag=f"ec{bt}{sfx}")
                nc.scalar.activation(ecomb, accs[t][:, 0:bt], func=act.Exp,
                                     scale=1.0 / WSCALE, bias=bb_sb)
                # transpose to batch-major [128 batch, j, 0:64|64:128] in
                # one [128,128] transpose per j-block. Tiles 0-3 stage fp16
                # (half the PE transpose cost); tiles 4/5 stage fp32 in the
                # long-retired acc0/acc1 banks so they never wait on the
                # 2-deep pstr rotation.
                if t < 4:
                    ps = pstr.tile([128, 4, 128], f16, tag="ps",
                                   name=f"ps{t}")
                    psC = [ps[:, j, :] for j in range(njs)]
                    psA = [ps[:, j, 0:64] for j in range(njs)]
                    psAall = ps[:, 0:njs, 0:64]
                    psBall = ps[:, 0:njs, 64:128]
                    idt = ident
                else:
                    psC = [accs[t - 4][:, j * 128:(j + 1) * 128]
                           for j in range(njs)]
                    psA = [accs[t - 4][:, j * 128:j * 128 + 64]
                           for j in range(njs)]
                    psAall = psA[0]
                    psBall = accs[t - 4][:, 64:128]
                    idt = ident32
                for j in range(njs):
                    nc.tensor.transpose(psC[j],
                                        ecomb[:, j * 128:(j + 1) * 128],
                                        idt)
                # den = 1 + eB; sigmoid = 1/den -- emitted before tv/zred so
                # the sig chain (which es and the final select depend on)
                # clears DVE first
                den = small.tile([128, 4, 64], ed, tag=f"den{sfx}")
                nc.vector.tensor_scalar_add(den[:, 0:njs, :], psBall, 1.0)
                sig = small.tile([128, 4, 64], ed, tag=f"sig{sfx}")
                with nc.allow_low_precision(reason="sigmoid in (0,1): fp16 "
                                            "rel err ~5e-4 vs 1.3e-2 l2"):
                    nc.vector.reciprocal(sig[:, 0:njs, :], den[:, 0:njs, :])
                # top-8 on exp(v) (monotone => same selection as on v);
                # zsum = sum of the top-8 values in ONE reduce over tv
                tv = tvp.tile([128, 32], ed, tag=f"tv{sfx}", name=f"tv{t}")
                for j in range(njs):
                    nc.vector.max(tv[:, j * 8:(j + 1) * 8], psA[j])
                zsum = small.tile([128, 4], f32, tag="zsum")
                nc.vector.tensor_reduce(
                    zsum[:, 0:njs],
                    tv.rearrange("p (j k) -> p j k", k=8)[:, 0:njs, :],
                    axis=mybir.AxisListType.X, op=alu.add)
                # es = e * sigmoid in ONE strided op. All ps reads stay on
                # DVE so the PSUM staging buffer frees as soon as the DVE
                # chain drains.
                es = small.tile([128, 4, 64], ed, tag=f"es{sfx}")
                nc.vector.tensor_mul(es[:, 0:njs, :], psAall,
                                     sig[:, 0:njs, :])
                # s4 = sum of top-8 e*sigmoid (accumulating select)
                s4 = small.tile([128, 4], f32, tag="s4")
                scr = small.tile([128, 4, 64], ed, tag=f"scr{sfx}")
                for j in range(njs):
                    nc.vector.scalar_tensor_tensor(
                        out=scr[:, j, :], in0=psA[j],
                        scalar=tv[:, j * 8 + 7:j * 8 + 8], in1=es[:, j, :],
                        op0=alu.is_ge, op1=alu.mult,
                        accum_out=s4[:, j:j + 1])
                rz = small.tile([128, 4], f32, tag="rz")
                nc.vector.reciprocal(rz[:, 0:njs], zsum[:, 0:njs])
                fin = small.tile([128, 4], f32, tag="fin")
                nc.vector.tensor_mul(fin[:, 0:njs], s4[:, 0:njs],
                                     rz[:, 0:njs])
                # outputs ride the gpsimd SWDGE queue mid-stream; the last
                # tile uses SP, idle once the x stream is issued.
                out_t = out[off:off + bt, :].rearrange(
                    "(j p) o -> p (j o)", j=njs, p=128)
                eng = nc.sync if t == len(TILES) - 1 else nc.gpsimd
                eng.dma_start(out=out_t, in_=fin[:, 0:njs])

    nc.compile()
    return nc


def get_program():
    if "prog" not in _cached:
        _cached["prog"] = _build_program()
    return _cached["prog"]


def make_in_maps(x, noise_w, noise_b, expert_w, expert_b):
    """Host-side sharding: per-core transposed fp8(e3m4) x + weights.

    The expert half is NEGATED (weights and bias) so the kernel computes
    exp(+scale*acc + bias) for all 128 logit rows in one ACT op:
    rows 64:128 then hold exp(-(expert_logit + expert_b)) directly.
    """
    import ml_dtypes
    w_comb = np.concatenate([noise_w, -expert_w], axis=0).astype(np.float32)
    wt32 = np.ascontiguousarray(w_comb.T) * np.float32(WSCALE)   # [D, 128]
    # partition p holds [nk, 128] for contraction rows nk*128+p
    wt = np.ascontiguousarray(
        wt32.reshape(NK, 128, 128).transpose(1, 0, 2).reshape(128, -1)
    ).astype(ml_dtypes.float8_e3m4)
    wt0 = np.ascontiguousarray(wt[:, :8 * 128])
    wt1 = np.ascontiguousarray(wt[:, 8 * 128:])
    bb = np.concatenate([noise_b, -expert_b]).reshape(128, 1).astype(
        np.float32)
    in_maps = []
    for c in range(NCORES):
        xs = np.ascontiguousarray(x[c * BC:(c + 1) * BC, :].T).astype(
            ml_dtypes.float8_e3m4)
        # per tile: [D, bt] -> [128, NK, bt], concatenated flat
        blocks = []
        for t, bt in enumerate(TILES):
            blk = xs[:, OFFS[t]:OFFS[t] + bt].reshape(NK, 128, bt)
            blocks.append(blk.transpose(1, 0, 2).reshape(-1))
        xr = np.ascontiguousarray(np.concatenate(blocks))
        in_maps.append({"xt": xr, "wt0": wt0, "wt1": wt1, "bb": bb})
    return in_maps


def kernel(x, noise, router_w, router_b, noise_w, noise_b, expert_w, expert_b,
           _trace=False):
    from concourse.bass_utils import run_bass_kernel_spmd

    x = np.asarray(x, dtype=np.float32)
    nc = get_program()
    in_maps = make_in_maps(x, np.asarray(noise_w), np.asarray(noise_b),
                           np.asarray(expert_w), np.asarray(expert_b))
    res = run_bass_kernel_spmd(nc, in_maps, core_ids=list(range(NCORES)),
                               trace=_trace)
    out = np.concatenate([r["out"] for r in res.results], axis=0)
    if _trace:
        kernel.last_results = res
    return out
